# revision 59
# baseline (speedup 1.0000x reference)
"""GAT 2-layer message-passing network on 8 TRN2 NeuronCores (Bass/Tile).

v3: restructured around the v2 trace findings (phase A Sync-issue-bound,
phases B/C gather-DGE + small-op bound, 337us repack of tiny descriptors).

Strategy (dst-sharded, uniform NPC=12544 with tail pad nodes):
 - Core c owns nodes [c*12544, (c+1)*12544) (core 7 has 352 pad nodes) and
   all real (non-self-loop) edges into them. Self loops are handled
   analytically on-chip (diagonal add), NOT via gather slots -- this cuts
   slot padding sharply.
 - Each core computes h only for its OWN nodes (mini-pass, 7 slab loads /
   stores with 128 large descriptors each, local pi rows p*98+b), then one
   AllGather replicates hloc into the Shared table htabS [100352, 768B] at
   rows pi(n) = c*12544 + (nl%128)*98 + nl//128. The layer-2 table h2tab64
   [100352, 256B] uses the same pi, so BOTH edge phases share one slot
   geometry, one gather-chunk function q = src//25088 (int16-safe indices),
   and one index array; only the table/row size differ.
 - Slots: per superblock (4 dst blocks) x chunk runs, tiles of 128 slots may
   span blocks; boundary tiles get one one-hot column-set per touched block
   (dloc sentinel 255 masks foreign slots), so padding is per-(sb,q) only.
 - Per sb: gather 768B rows; a_dst per slot via oT one-hot matmuls from
   SBUF-resident slocS; ex=exp(lrelu(asrc+adst)); msg in-place; per-block
   PSUM aggregation via oh one-hot matmuls; self-loop contribution added as
   vector ops from an hloc row load; batched (per-sb) normalize + bias +
   relu + W2 matmul; h2 rows staged and stored in pi_C layout.
 - AllGather h2loc64 [12544,64]f32 -> Shared h2tab64 [100352,64].
 - Phase C: same slots, 256B-row gathers, batched epilogue into vstage;
   single final log-softmax over all blocks and one pi-ordered output store.
"""
import sys

if "/opt/trn_rl_repo" not in sys.path:
    sys.path.insert(0, "/opt/trn_rl_repo")

import math
import numpy as np
import ml_dtypes

import concourse.bass as bass
import concourse.bacc as bacc
import concourse.mybir as mybir
import concourse.tile as tile
from concourse import bass_utils

P = 128
NEG = 0.2
NCHUNK = 4
NQUEUE = 4
SLAB = 14                 # phase-A tiles per slab (14 | 196)
MAXT = 7                  # tiles per dma_gather call (1024-desc rings)
DMA_SCRATCH = 16384       # SWDGE carveout bytes/partition (1024 descs/queue)

# Tile's DMASW sem-lane assignment round-robins over all Pool DMAs, which
# breaks the per-lane FIFO assumption when SWDGE DMAs run on multiple queues
# (out-of-order completion across queues under one counting sem). Patch the
# lane choice to lane == queue_num: per-lane FIFO again holds (each HW ring
# drains in order), and queues get independent lanes.
from concourse import tile_sem_assignment as _tsa  # noqa: E402

if not getattr(_tsa.TileClockTick, "_qaware_patched", False):
    _orig_assign_tick = _tsa.TileClockTick._assign_tick

    def _qaware_assign_tick(self, inst):
        if (isinstance(inst, _tsa.DMAInst)
                and inst.engine == mybir.EngineType.Pool):
            self.next_sw_dma_idx = getattr(inst, "queue_num", 0) or 0
        return _orig_assign_tick(self, inst)

    _tsa.TileClockTick._assign_tick = _qaware_assign_tick
    _tsa.TileClockTick._qaware_patched = True


def _wrap16(flat):
    """[n] -> [128, n//16] wrapped in 16 partitions, replicated x8."""
    w = flat.reshape(-1, 16).T
    return np.tile(w, (8, 1))


# ----------------------------------------------------------------------------
# host-side data prep
# ----------------------------------------------------------------------------

def prep(inputs, cfg):
    N, F, H, C, CLS, NC = cfg["N"], cfg["F"], cfg["H"], cfg["C"], cfg["CLS"], cfg["NC"]
    SBG = cfg.get("SBG", 4)
    x = np.asarray(inputs["x"], np.float32)
    ei = np.asarray(inputs["edge_index"])
    W1 = np.asarray(inputs["W1"], np.float32)
    as1 = np.asarray(inputs["att_src1"], np.float32)
    ad1 = np.asarray(inputs["att_dst1"], np.float32)
    b1 = np.asarray(inputs["b1"], np.float32)
    W2 = np.asarray(inputs["W2"], np.float32)
    as2 = np.asarray(inputs["att_src2"], np.float32)
    ad2 = np.asarray(inputs["att_dst2"], np.float32)
    b2 = np.asarray(inputs["b2"], np.float32)

    HC = H * C                        # 256
    R1 = HC + H                       # gathered live row: [h | asrc]
    RG = 128 * math.ceil((R1 + H) / 128)  # 384 bf16 elems (768B rows)
    NPC, NB = 12544, 98
    NT = 784
    Np = NT * P                       # 100352
    CHB = Np // NCHUNK                # 25088 = 196*128 = 2*NPC
    TPC = CHB // P                    # 196 tiles per chunk
    RL2 = 64                          # f32 row elems for L2 table (256B)

    # ---- weights / constants -------------------------------------------------
    W1r = W1.reshape(F, H, C)
    Wsrc = np.einsum("fhc,hc->fh", W1r, as1)
    Wdst = np.einsum("fhc,hc->fh", W1r, ad1)
    W1aug = np.concatenate([W1, Wsrc, Wdst], axis=1)          # [F, 264]
    Wsrc2 = W2 @ as2.reshape(CLS, 1)
    Wdst2 = W2 @ ad2.reshape(CLS, 1)
    W2aug = np.concatenate([W2, Wsrc2, Wdst2], axis=1)        # [HC, 4]

    bf16 = ml_dtypes.bfloat16
    xT = np.zeros((F, Np), dtype=bf16)
    xT[:, :N] = x.T.astype(bf16)
    W1aug_b = W1aug.astype(bf16)
    W2aug_b = W2aug.astype(bf16)
    b1rep = np.tile(b1[None, :], (P, 1)).astype(bf16)
    b2rep = np.tile(b2[None, :], (P, 1)).astype(np.float32)
    iota = np.tile(np.arange(P, dtype=np.float32)[None, :], (P, 1)).astype(bf16)
    ident = np.eye(P, dtype=bf16)

    # ---- edges (real only; self loops handled on-chip) -----------------------
    src_all = np.asarray(ei[0], np.int64)
    dst_all = np.asarray(ei[1], np.int64)
    order = np.argsort(dst_all, kind="stable")
    src_s = src_all[order]
    dst_s = dst_all[order]
    q_s = src_s // CHB                                        # phase chunk

    # superblocks of dst blocks
    sblocks = [list(range(i, min(i + SBG, NB))) for i in range(0, NB, SBG)]

    # per-core, per-(sb, q, block) counts
    nsb = len(sblocks)
    cnt = np.zeros((NC, nsb, NCHUNK, SBG), np.int64)
    for c in range(NC):
        for si, blist in enumerate(sblocks):
            for bi, b in enumerate(blist):
                lo = c * NPC + b * P
                lo_i, hi_i = np.searchsorted(dst_s, lo), np.searchsorted(dst_s, lo + P)
                qs = q_s[lo_i:hi_i]
                for q in range(NCHUNK):
                    cnt[c, si, q, bi] = (qs == q).sum()
    cnt_sq = cnt.sum(axis=3)                                  # [NC, nsb, q]
    Trun = np.ceil(cnt_sq / P).astype(np.int64).max(axis=0)   # [nsb, q]
    Trun = np.maximum(Trun, 1)

    # slot layout + instance structure (global, core-agnostic)
    sb_meta = []
    tile_base = 0
    oh_base = 0
    for si, blist in enumerate(sblocks):
        segs = []               # per q: (tile_base_global, T)
        sb_tb = tile_base
        sb_ohb = oh_base
        pad_groups = []         # per tile_rel: list of (ohcol_rel, bi)
        agg = {bi: [] for bi in range(len(blist))}   # bi -> [(tile_rel, ohcol_rel)]
        inst_desc = []          # (tile_rel, bi) in oh column order
        for q in range(NCHUNK):
            T = int(Trun[si, q])
            segs.append((tile_base, T))
            # instance structure: union over cores of block spans
            # block bi span in run for core c: [off[c][bi], off[c][bi+1])
            offs = np.zeros((NC, len(blist) + 1), np.int64)
            for c in range(NC):
                offs[c, 1:] = np.cumsum(cnt[c, si, q, :len(blist)])
            for t in range(T):
                t_rel_global = tile_base - sb_tb + t
                s0, s1 = t * P, (t + 1) * P
                for bi in range(len(blist)):
                    hit = False
                    for c in range(NC):
                        if offs[c, bi] < s1 and offs[c, bi + 1] > s0:
                            hit = True
                            break
                    if hit:
                        inst_desc.append((t_rel_global, bi))
            tile_base += T
        S = tile_base - sb_tb
        Sx = len(inst_desc)
        oh_base += Sx
        pad_groups = [[] for _ in range(S)]
        for ohc, (t_rel, bi) in enumerate(inst_desc):
            pad_groups[t_rel].append((ohc, bi))
            agg[bi].append((t_rel, ohc))
        sb_meta.append(dict(base=sb_tb, S=S, ohbase=sb_ohb, Sx=Sx, segs=segs,
                            blocks=blist, b0=blist[0], inst=inst_desc,
                            pad_groups=pad_groups, agg=agg))
    Tsum = tile_base
    SxT = oh_base

    # per-core slot-value arrays
    ihC_w = np.zeros((NC, P, Tsum * 8), np.int16)
    dlx2d = np.zeros((NC, P, SxT), bf16)
    dlxT = np.zeros((NC, 1, SxT * P), bf16)
    oTh = np.zeros((NC, P, SxT * P), bf16)
    for c in range(NC):
        ihC = np.zeros(Tsum * P, np.int16)
        dlx = np.full(SxT * P, 255.0, np.float32)
        for si, blist in enumerate(sblocks):
            sb = sb_meta[si]
            for q in range(NCHUNK):
                tb, T = sb["segs"][q]
                # this core's edges for (sb, q), dst-sorted
                lo = c * NPC + blist[0] * P
                hi = c * NPC + blist[-1] * P + P
                lo_i, hi_i = np.searchsorted(dst_s, lo), np.searchsorted(dst_s, hi)
                m = q_s[lo_i:hi_i] == q
                es = src_s[lo_i:hi_i][m]
                ed = dst_s[lo_i:hi_i][m]
                n = len(es)
                assert n <= T * P, (n, T * P)
                s0 = tb * P
                # row idx within chunk q (pi_C layout, used by both phases)
                cs = es // NPC
                loc = es % NPC
                ihC[s0:s0 + n] = ((cs % 2) * NPC + (loc % P) * NB
                                  + loc // P).astype(np.int16)
                # dloc per instance column
                blk = (ed - c * NPC) // P - blist[0]          # bi of each edge
                dloc = ed - (c * NPC + (blist[0] + blk) * P)  # 0..127
                for ohc, (t_rel, bi) in enumerate(sb["inst"]):
                    pass
                # fill instance columns for this (sb, q)
                for t in range(T):
                    t_rel = tb - sb["base"] + t
                    e0, e1 = t * P, min((t + 1) * P, n)
                    if e0 >= n:
                        continue
                    for (ohc, bi) in sb["pad_groups"][t_rel]:
                        col0 = (sb["ohbase"] + ohc) * P
                        idx = np.arange(e0, e1)
                        sel = blk[idx] == bi
                        lanes = idx - t * P
                        vals = np.full(len(idx), 255.0, np.float32)
                        vals[sel] = dloc[idx[sel]]
                        dlx[col0 + lanes] = vals
        ihC_w[c] = _wrap16(ihC)
        dlx2d[c] = dlx.reshape(SxT, P).T.astype(bf16)
        dlxT[c, 0] = dlx.astype(bf16)
        oTh[c] = (np.arange(P, dtype=np.float32)[:, None]
                  == dlx[None, :]).astype(bf16)

    shared = {
        "W1aug": W1aug_b, "W2aug": W2aug_b, "b1rep": b1rep,
        "b2rep": b2rep, "iota": iota, "ident": ident,
        "iotac": np.arange(P, dtype=np.float32).reshape(P, 1),
        "onesk": np.ones((1, P), bf16),
    }
    in_maps = []
    for c in range(NC):
        m = dict(shared)
        m["xTloc"] = np.ascontiguousarray(xT[:, c * NPC:(c + 1) * NPC])
        m["ihsrcC"] = ihC_w[c]
        m["dlx2d"] = dlx2d[c]
        m["dlxT"] = dlxT[c]
        m["oTh"] = oTh[c]
        in_maps.append(m)

    meta = dict(cfg, R1=R1, RG=RG, HC=HC, NPC=NPC, NB=NB, NT=NT, Np=Np,
                CHB=CHB, TPC=TPC, RL2=RL2, Tsum=Tsum, SxT=SxT,
                sb_meta=sb_meta, SBG=SBG)
    return in_maps, meta


# ----------------------------------------------------------------------------
# device program
# ----------------------------------------------------------------------------

def _sub(ap, elem_off, dims):
    return bass.AP(ap.tensor, ap.offset + elem_off, [ap.ap[0], *list(dims)])


def build(meta, nc=None):
    N, F, H, C, CLS = meta["N"], meta["F"], meta["H"], meta["C"], meta["CLS"]
    NC, R1, RG, HC = meta["NC"], meta["R1"], meta["RG"], meta["HC"]
    NPC, NB, NT, Np = meta["NPC"], meta["NB"], meta["NT"], meta["Np"]
    CHB, TPC, RL2 = meta["CHB"], meta["TPC"], meta["RL2"]
    Tsum, SxT = meta["Tsum"], meta["SxT"]
    sb_meta = meta["sb_meta"]
    SBG = meta["SBG"]
    R2 = 4

    f32, bf16, i16 = mybir.dt.float32, mybir.dt.bfloat16, mybir.dt.int16

    if nc is None:
        nc = bacc.Bacc("TRN2", target_bir_lowering=False, debug=False,
                       num_devices=NC, num_swdge_queues=NQUEUE,
                       dynamic_dma_scratch_size=DMA_SCRATCH)

    qrr = [0]

    def gather_split(out_tile, rel, segT, elem, table, ix_tile):
        """Split a segment gather into <=MAXT-tile calls, round-robin queues."""
        done = 0
        while done < segT:
            tt = min(MAXT, segT - done)
            r = rel + done
            nc.gpsimd.dma_gather(
                bass.AP(out_tile[:].tensor, out_tile[:].offset + r * elem,
                        [out_tile[:].ap[0], [elem, tt], [1, elem]]),
                table,
                ix_tile[:, r * 8:(r + tt) * 8],
                tt * P, tt * P, elem,
                queue_num=qrr[0] % NQUEUE,
            )
            qrr[0] += 1
            done += tt

    xTl_d = nc.dram_tensor("xTloc", [F, NPC], bf16, kind="ExternalInput")
    W1aug_d = nc.dram_tensor("W1aug", [F, R1 + H], bf16, kind="ExternalInput")
    W2aug_d = nc.dram_tensor("W2aug", [HC, R2], bf16, kind="ExternalInput")
    b1rep_d = nc.dram_tensor("b1rep", [P, HC], bf16, kind="ExternalInput")
    b2rep_d = nc.dram_tensor("b2rep", [P, CLS], f32, kind="ExternalInput")
    iota_d = nc.dram_tensor("iota", [P, P], bf16, kind="ExternalInput")
    ident_d = nc.dram_tensor("ident", [P, P], bf16, kind="ExternalInput")
    ihC_d = nc.dram_tensor("ihsrcC", [P, Tsum * 8], i16, kind="ExternalInput")
    dlx2d_d = nc.dram_tensor("dlx2d", [P, SxT], bf16, kind="ExternalInput")
    dlxT_d = nc.dram_tensor("dlxT", [1, SxT * P], bf16, kind="ExternalInput")
    oTh_d = nc.dram_tensor("oTh", [P, SxT * P], bf16, kind="ExternalInput")
    iotac_d = nc.dram_tensor("iotac", [P, 1], f32, kind="ExternalInput")
    onesk_d = nc.dram_tensor("onesk", [1, P], bf16, kind="ExternalInput")
    out_d = nc.dram_tensor("out", [NPC, CLS], f32, kind="ExternalOutput")

    hloc = nc.dram_tensor("hloc", [NPC, RG], bf16, kind="Internal")
    htabS = nc.dram_tensor("htabS", [Np, RG], bf16, kind="Internal",
                           addr_space="Shared")
    h2loc64 = nc.dram_tensor("h2loc64", [NPC, RL2], f32, kind="Internal")
    h2tab64 = nc.dram_tensor("h2tab64", [Np, RL2], f32, kind="Internal",
                             addr_space="Shared")

    FA = min(P, F)
    FB = F - FA

    with tile.TileContext(nc) as tc:
        with tc.tile_pool(name="const", bufs=1) as cp:
            w1a = cp.tile([FA, R1 + H], bf16)
            nc.sync.dma_start(out=w1a[:], in_=W1aug_d[0:FA, :])
            w1b = cp.tile([FB, R1 + H], bf16)
            nc.sync.dma_start(out=w1b[:], in_=W1aug_d[FA:F, :])
            w2a = cp.tile([P, R2], bf16)
            nc.sync.dma_start(out=w2a[:], in_=W2aug_d[0:P, :])
            w2b = cp.tile([P, R2], bf16)
            nc.sync.dma_start(out=w2b[:], in_=W2aug_d[P:HC, :])
            b1s = cp.tile([P, HC], bf16)
            nc.sync.dma_start(out=b1s[:], in_=b1rep_d[:, :])
            b2s = cp.tile([P, CLS], f32)
            nc.sync.dma_start(out=b2s[:], in_=b2rep_d[:, :])
            iot = cp.tile([P, P], bf16)
            nc.sync.dma_start(out=iot[:], in_=iota_d[:, :])
            idn = cp.tile([P, P], bf16)
            nc.sync.dma_start(out=idn[:], in_=ident_d[:, :])
            dlc = cp.tile([P, SxT], bf16)
            nc.sync.dma_start(out=dlc[:], in_=dlx2d_d[:, :])
            iotc = cp.tile([P, 1], f32)
            nc.sync.dma_start(out=iotc[:], in_=iotac_d[:, :])
            onek = cp.tile([1, P], bf16)
            nc.sync.dma_start(out=onek[:], in_=onesk_d[:, :])
            # persistent SBUF state
            slocS = cp.tile([P, NB * H], bf16)       # a_dst of own nodes
            h2self = cp.tile([P, NB * R2], f32)      # own h2 rows
            vstage = cp.tile([P, NB * R2], f32)      # phase-C accumulators
            padAll = cp.tile([P, Tsum * H], bf16)    # per-slot a_dst (exact bf16)
            pad2All = cp.tile([P, Tsum], bf16)       # per-slot a_dst2

            # ---------------- mini-pass: h table (own slice) -----------------
            with tc.tile_pool(name="pm", bufs=2) as pm, \
                 tc.tile_pool(name="psm", bufs=4, space="PSUM") as psm:
                for s in range(NPC // (SLAB * P)):          # 7 slabs
                    c0 = s * SLAB * P
                    xa = pm.tile([FA, SLAB * P], bf16, tag="xa")
                    nc.sync.dma_start(out=xa[:], in_=xTl_d[0:FA, c0:c0 + SLAB * P])
                    xb = pm.tile([FB, SLAB * P], bf16, tag="xb")
                    nc.sync.dma_start(out=xb[:], in_=xTl_d[FA:F, c0:c0 + SLAB * P])
                    stg = pm.tile([P, SLAB * RG], bf16, tag="stg")
                    for t in range(SLAB):
                        ph = psm.tile([P, R1 + H], f32, tag="ph")
                        nc.tensor.matmul(out=ph[:], lhsT=xa[:, t * P:(t + 1) * P],
                                         rhs=w1a[:], start=True, stop=False)
                        nc.tensor.matmul(out=ph[:], lhsT=xb[:, t * P:(t + 1) * P],
                                         rhs=w1b[:], start=False, stop=True)
                        nc.vector.tensor_copy(out=stg[:, t * RG:t * RG + R1],
                                              in_=ph[:, :R1])
                        nc.vector.tensor_copy(
                            out=slocS[:, (s * SLAB + t) * H:(s * SLAB + t + 1) * H],
                            in_=ph[:, R1:R1 + H])
                    nc.sync.dma_start(
                        out=bass.AP(hloc, s * SLAB * RG,
                                    [[NB * RG, P], [RG, SLAB], [1, RG]]),
                        in_=stg[:])

            # ---------------- AllGather h table ------------------------------
            nc.gpsimd.collective_compute(
                "AllGather", mybir.AluOpType.bypass,
                replica_groups=[list(range(NC))],
                ins=[hloc[:, :]], outs=[htabS[:, :]])

            # ---------------- pad-pass: per-slot a_dst (overlaps the AG) -----
            # oT(bf16 one-hot) @ slocS(bf16) selects one bf16 value -> the f32
            # PSUM result is exactly representable in bf16: lossless stash.
            with tc.tile_pool(name="pp", bufs=2) as ppool, \
                 tc.tile_pool(name="psq", bufs=2, space="PSUM") as psq:
                for sb in sb_meta:
                    base, S, Sx, ohb = sb["base"], sb["S"], sb["Sx"], sb["ohbase"]
                    b0 = sb["b0"]
                    oTp = ppool.tile([P, 60 * P], bf16, tag="oTp")
                    h1 = (Sx // 2) * P
                    nc.sync.dma_start(out=oTp[:, :h1],
                                      in_=oTh_d[:, ohb * P:ohb * P + h1])
                    nc.sync.dma_start(out=oTp[:, h1:Sx * P],
                                      in_=oTh_d[:, ohb * P + h1:(ohb + Sx) * P])
                    padp = psq.tile([P, 160], f32, tag="padp")
                    for t in range(S):
                        grp = sb["pad_groups"][t]
                        for gi, (ohc, bi) in enumerate(grp):
                            nc.tensor.matmul(
                                out=padp[:, t * H:(t + 1) * H],
                                lhsT=oTp[:, ohc * P:(ohc + 1) * P],
                                rhs=slocS[:, (b0 + bi) * H:(b0 + bi + 1) * H],
                                start=(gi == 0), stop=(gi == len(grp) - 1),
                                skip_group_check=True)
                    nc.vector.tensor_copy(
                        out=padAll[:, base * H:(base + S) * H],
                        in_=padp[:, :S * H])

            # ---------------- Phase B: L1 edge pass --------------------------
            with tc.tile_pool(name="pbg", bufs=3) as pbg, \
                 tc.tile_pool(name="pbo", bufs=2) as pbo, \
                 tc.tile_pool(name="pbb", bufs=3) as pbb, \
                 tc.tile_pool(name="psb", bufs=4, space="PSUM") as psb, \
                 tc.tile_pool(name="pst", bufs=1, space="PSUM") as pst, \
                 tc.tile_pool(name="psh", bufs=1, space="PSUM") as psh:
                for sb in sb_meta:
                    base, S, Sx, ohb = sb["base"], sb["S"], sb["Sx"], sb["ohbase"]
                    blist, b0 = sb["blocks"], sb["b0"]
                    nblk = len(blist)
                    g = pbg.tile([P, S * RG], bf16, tag="g")
                    ixs = pbg.tile([P, S * 8], i16, tag="ixs")
                    nc.sync.dma_start(out=ixs[:],
                                      in_=ihC_d[:, base * 8:(base + S) * 8])
                    for q in range(NCHUNK):
                        tb, T = sb["segs"][q]
                        gather_split(g, tb - base, T, RG,
                                     htabS[q * CHB:(q + 1) * CHB, :], ixs)
                    selfh = pbg.tile([P, SBG * RG], bf16, tag="selfh")
                    nc.sync.dma_start(
                        out=selfh[:, :nblk * RG],
                        in_=bass.AP(hloc, b0 * RG,
                                    [[NB * RG, P], [1, nblk * RG]]))
                    # ex = exp(lrelu(asrc+adst))  [P, S*H] f32
                    ex = pbb.tile([P, S * H], f32, tag="ex")
                    nc.vector.tensor_tensor(
                        out=ex[:].rearrange("p (t h) -> p t h", t=S),
                        in0=_sub(g[:], HC, [[RG, S], [1, H]]),
                        in1=_sub(padAll[:], base * H, [[H, S], [1, H]]),
                        op=mybir.AluOpType.add)
                    tmp = pbb.tile([P, S * H], f32, tag="tmp")
                    nc.vector.tensor_scalar_mul(out=tmp[:], in0=ex[:], scalar1=NEG)
                    nc.vector.tensor_tensor(out=ex[:], in0=ex[:], in1=tmp[:],
                                            op=mybir.AluOpType.max)
                    nc.scalar.activation(out=ex[:], in_=ex[:],
                                         func=mybir.ActivationFunctionType.Exp)
                    exb = pbb.tile([P, S * H], bf16, tag="exb")
                    nc.vector.tensor_copy(out=exb[:], in_=ex[:])
                    # msg in-place: cols 0:HC *= ex ; cols HC:HC+H = ex
                    nc.vector.tensor_tensor(
                        out=_sub(g[:], 0, [[RG, S], [C, H], [1, C]]),
                        in0=_sub(g[:], 0, [[RG, S], [C, H], [1, C]]),
                        in1=_sub(exb[:], 0, [[H, S], [1, H], [0, C]]),
                        op=mybir.AluOpType.mult)
                    nc.vector.tensor_copy(
                        out=_sub(g[:], HC, [[RG, S], [1, H]]),
                        in_=exb[:].rearrange("p (t h) -> p t h", t=S))
                    # oh: [slot, inst_col] one-hot
                    oh = pbo.tile([P, Sx * P], bf16, tag="oh")
                    nc.vector.tensor_tensor(
                        out=oh[:].rearrange("p (t q) -> p t q", t=Sx),
                        in0=_sub(dlc[:], ohb, [[1, Sx], [0, P]]),
                        in1=_sub(iot[:], 0, [[0, Sx], [1, P]]),
                        op=mybir.AluOpType.is_equal)
                    # self-loop messages (batched over blocks)
                    lS = pbb.tile([P, SBG * H], f32, tag="lS")
                    nc.vector.tensor_tensor(
                        out=lS[:, :nblk * H].rearrange("p (b h) -> p b h", b=nblk),
                        in0=_sub(selfh[:], HC, [[RG, nblk], [1, H]]),
                        in1=_sub(slocS[:], b0 * H, [[H, nblk], [1, H]]),
                        op=mybir.AluOpType.add)
                    tS = pbb.tile([P, SBG * H], f32, tag="tS")
                    nc.vector.tensor_scalar_mul(out=tS[:], in0=lS[:], scalar1=NEG)
                    nc.vector.tensor_tensor(out=lS[:], in0=lS[:], in1=tS[:],
                                            op=mybir.AluOpType.max)
                    nc.scalar.activation(out=lS[:], in_=lS[:],
                                         func=mybir.ActivationFunctionType.Exp)
                    selfm = pbb.tile([P, SBG * (R1 + 4)], bf16, tag="selfm")
                    RS = R1 + 4
                    nc.vector.tensor_tensor(
                        out=_sub(selfm[:], 0, [[RS, nblk], [C, H], [1, C]]),
                        in0=_sub(selfh[:], 0, [[RG, nblk], [C, H], [1, C]]),
                        in1=_sub(lS[:], 0, [[H, nblk], [1, H], [0, C]]),
                        op=mybir.AluOpType.mult)
                    nc.vector.tensor_copy(
                        out=_sub(selfm[:], HC, [[RS, nblk], [1, H]]),
                        in_=lS[:, :nblk * H].rearrange("p (b h) -> p b h", b=nblk))
                    # per-block aggregation + batched epilogue
                    psoS = pbb.tile([P, SBG * RS], f32, tag="psoS")
                    for bi in range(nblk):
                        runs = sb["agg"][bi]
                        pso = psb.tile([P, R1], f32, tag="pso")
                        for ri, (t_rel, ohc) in enumerate(runs):
                            nc.tensor.matmul(
                                out=pso[:],
                                lhsT=oh[:, ohc * P:(ohc + 1) * P],
                                rhs=g[:, t_rel * RG:t_rel * RG + R1],
                                start=(ri == 0), stop=(ri == len(runs) - 1))
                        nc.vector.tensor_tensor(
                            out=psoS[:, bi * RS:bi * RS + R1],
                            in0=pso[:],
                            in1=selfm[:, bi * RS:bi * RS + R1],
                            op=mybir.AluOpType.add)
                    den = pbb.tile([P, SBG * H], f32, tag="den")
                    nc.vector.tensor_scalar_max(
                        out=den[:, :nblk * H].rearrange("p (b h) -> p b h", b=nblk),
                        in0=_sub(psoS[:], HC, [[RS, nblk], [1, H]]),
                        scalar1=1e-20)
                    rde = pbb.tile([P, SBG * H], f32, tag="rde")
                    nc.vector.reciprocal(out=rde[:], in_=den[:])
                    o1 = pbb.tile([P, SBG * HC], bf16, tag="o1")
                    nc.vector.tensor_tensor(
                        out=o1[:].rearrange("p (b h c) -> p b h c", b=SBG, h=H),
                        in0=_sub(psoS[:], 0, [[RS, SBG], [C, H], [1, C]]),
                        in1=_sub(rde[:], 0, [[H, SBG], [1, H], [0, C]]),
                        op=mybir.AluOpType.mult)
                    nc.vector.tensor_tensor(
                        out=o1[:].rearrange("p (b k) -> p b k", b=SBG),
                        in0=o1[:].rearrange("p (b k) -> p b k", b=SBG),
                        in1=_sub(b1s[:], 0, [[0, SBG], [1, HC]]),
                        op=mybir.AluOpType.add)
                    nc.scalar.activation(out=o1[:], in_=o1[:],
                                         func=mybir.ActivationFunctionType.Relu)
                    # h2 = relu(o1) @ W2aug via PE transposes
                    ptr = pst.tile([P, 2 * SBG * P], bf16, tag="ptr")
                    for bi in range(nblk):
                        for k in range(2):
                            nc.tensor.transpose(
                                out=ptr[:, (bi * 2 + k) * P:(bi * 2 + k + 1) * P],
                                in_=o1[:, bi * HC + k * P:bi * HC + (k + 1) * P],
                                identity=idn[:])
                    rT = pbb.tile([P, 2 * SBG * P], bf16, tag="rT")
                    nc.vector.tensor_copy(out=rT[:, :nblk * 2 * P],
                                          in_=ptr[:, :nblk * 2 * P])
                    ph2 = psh.tile([P, SBG * R2], f32, tag="ph2")
                    for bi in range(nblk):
                        nc.tensor.matmul(out=ph2[:, bi * R2:(bi + 1) * R2],
                                         lhsT=rT[:, bi * 2 * P:(bi * 2 + 1) * P],
                                         rhs=w2a[:], start=True, stop=False,
                                         skip_group_check=True)
                        nc.tensor.matmul(out=ph2[:, bi * R2:(bi + 1) * R2],
                                         lhsT=rT[:, (bi * 2 + 1) * P:(bi * 2 + 2) * P],
                                         rhs=w2b[:], start=False, stop=True,
                                         skip_group_check=True)
                    nc.vector.tensor_copy(out=h2self[:, b0 * R2:(b0 + nblk) * R2],
                                          in_=ph2[:, :nblk * R2])
                    h2st = pbb.tile([P, SBG * RL2], f32, tag="h2st")
                    nc.vector.tensor_copy(
                        out=_sub(h2st[:], 0, [[RL2, nblk], [1, R2]]),
                        in_=ph2[:, :nblk * R2].rearrange("p (b r) -> p b r", b=nblk))
                    nc.sync.dma_start(
                        out=bass.AP(h2loc64, b0 * RL2,
                                    [[NB * RL2, P], [1, nblk * RL2]]),
                        in_=h2st[:, :nblk * RL2])

            # ---------------- AllGather ---------------------------------------
            nc.gpsimd.collective_compute(
                "AllGather", mybir.AluOpType.bypass,
                replica_groups=[list(range(NC))],
                ins=[h2loc64[:, :]], outs=[h2tab64[:, :]])

            # ---------------- pad2-pass: per-slot a_dst2 (overlaps the AG) ---
            with tc.tile_pool(name="pp2", bufs=2) as pp2, \
                 tc.tile_pool(name="psq2", bufs=2, space="PSUM") as psq2:
                for sb in sb_meta:
                    base, S, Sx, ohb = sb["base"], sb["S"], sb["Sx"], sb["ohbase"]
                    blist, b0 = sb["blocks"], sb["b0"]
                    nblk = len(blist)
                    oTp = pp2.tile([P, 60 * P], bf16, tag="oTp2")
                    h1 = (Sx // 2) * P
                    nc.sync.dma_start(out=oTp[:, :h1],
                                      in_=oTh_d[:, ohb * P:ohb * P + h1])
                    nc.sync.dma_start(out=oTp[:, h1:Sx * P],
                                      in_=oTh_d[:, ohb * P + h1:(ohb + Sx) * P])
                    adw2 = pp2.tile([P, SBG], bf16, tag="adw2p")
                    nc.vector.tensor_copy(
                        out=adw2[:, :nblk],
                        in_=_sub(h2self[:], b0 * R2 + 3, [[R2, nblk]]))
                    pad2p = psq2.tile([P, 64], f32, tag="pad2p")
                    for t in range(S):
                        grp = sb["pad_groups"][t]
                        for gi, (ohc, bi) in enumerate(grp):
                            nc.tensor.matmul(
                                out=pad2p[:, t:t + 1],
                                lhsT=oTp[:, ohc * P:(ohc + 1) * P],
                                rhs=adw2[:, bi:bi + 1],
                                start=(gi == 0), stop=(gi == len(grp) - 1),
                                skip_group_check=True)
                    nc.vector.tensor_copy(
                        out=pad2All[:, base:base + S],
                        in_=pad2p[:, :S])

            # ---------------- Phase C: L2 edge pass --------------------------
            with tc.tile_pool(name="pcg", bufs=4) as pcg, \
                 tc.tile_pool(name="pco", bufs=3) as pco, \
                 tc.tile_pool(name="pcb", bufs=2) as pcb, \
                 tc.tile_pool(name="psc", bufs=4, space="PSUM") as psc:
                for sb in sb_meta:
                    base, S, Sx, ohb = sb["base"], sb["S"], sb["Sx"], sb["ohbase"]
                    blist, b0 = sb["blocks"], sb["b0"]
                    nblk = len(blist)
                    g2 = pcg.tile([P, S * RL2], f32, tag="g2")
                    ixs = pcg.tile([P, S * 8], i16, tag="ixs2")
                    nc.sync.dma_start(out=ixs[:],
                                      in_=ihC_d[:, base * 8:(base + S) * 8])
                    for q in range(NCHUNK):
                        tb, T = sb["segs"][q]
                        gather_split(g2, tb - base, T, RL2,
                                     h2tab64[q * CHB:(q + 1) * CHB, :], ixs)
                    ex2 = pcb.tile([P, S], f32, tag="ex2")
                    nc.vector.tensor_tensor(
                        out=ex2[:],
                        in0=_sub(g2[:], CLS, [[RL2, S]]),
                        in1=_sub(pad2All[:], base, [[1, S]]),
                        op=mybir.AluOpType.add)
                    tm2 = pcb.tile([P, S], f32, tag="tm2")
                    nc.vector.tensor_scalar_mul(out=tm2[:], in0=ex2[:], scalar1=NEG)
                    nc.vector.tensor_tensor(out=ex2[:], in0=ex2[:], in1=tm2[:],
                                            op=mybir.AluOpType.max)
                    nc.scalar.activation(out=ex2[:], in_=ex2[:],
                                         func=mybir.ActivationFunctionType.Exp)
                    m2 = pcb.tile([P, S * R2], bf16, tag="m2")
                    nc.vector.tensor_tensor(
                        out=_sub(m2[:], 0, [[R2, S], [1, CLS]]),
                        in0=_sub(g2[:], 0, [[RL2, S], [1, CLS]]),
                        in1=_sub(ex2[:], 0, [[1, S], [0, CLS]]),
                        op=mybir.AluOpType.mult)
                    nc.vector.tensor_copy(
                        out=_sub(m2[:], CLS, [[R2, S], [1, 2]]),
                        in_=_sub(ex2[:], 0, [[1, S], [0, 2]]))
                    oh = pco.tile([P, Sx * P], bf16, tag="oh2")
                    nc.vector.tensor_tensor(
                        out=oh[:].rearrange("p (t q) -> p t q", t=Sx),
                        in0=_sub(dlc[:], ohb, [[1, Sx], [0, P]]),
                        in1=_sub(iot[:], 0, [[0, Sx], [1, P]]),
                        op=mybir.AluOpType.is_equal)
                    # self-loop L2 messages
                    l2S = pcb.tile([P, SBG], f32, tag="l2S")
                    nc.vector.tensor_tensor(
                        out=l2S[:, :nblk],
                        in0=_sub(h2self[:], b0 * R2 + 2, [[R2, nblk]]),
                        in1=_sub(h2self[:], b0 * R2 + 3, [[R2, nblk]]),
                        op=mybir.AluOpType.add)
                    t2S = pcb.tile([P, SBG], f32, tag="t2S")
                    nc.vector.tensor_scalar_mul(out=t2S[:], in0=l2S[:], scalar1=NEG)
                    nc.vector.tensor_tensor(out=l2S[:], in0=l2S[:], in1=t2S[:],
                                            op=mybir.AluOpType.max)
                    nc.scalar.activation(out=l2S[:], in_=l2S[:],
                                         func=mybir.ActivationFunctionType.Exp)
                    sm2 = pcb.tile([P, SBG * R2], f32, tag="sm2")
                    nc.vector.tensor_tensor(
                        out=_sub(sm2[:], 0, [[R2, nblk], [1, CLS]]),
                        in0=_sub(h2self[:], b0 * R2, [[R2, nblk], [1, CLS]]),
                        in1=_sub(l2S[:], 0, [[1, nblk], [0, CLS]]),
                        op=mybir.AluOpType.mult)
                    nc.vector.tensor_copy(
                        out=_sub(sm2[:], CLS, [[R2, nblk], [1, 2]]),
                        in_=_sub(l2S[:], 0, [[1, nblk], [0, 2]]))
                    ps2 = psc.tile([P, SBG * R2], f32, tag="ps2")
                    for bi in range(nblk):
                        runs = sb["agg"][bi]
                        for ri, (t_rel, ohc) in enumerate(runs):
                            nc.tensor.matmul(
                                out=ps2[:, bi * R2:(bi + 1) * R2],
                                lhsT=oh[:, ohc * P:(ohc + 1) * P],
                                rhs=m2[:, t_rel * R2:(t_rel + 1) * R2],
                                start=(ri == 0), stop=(ri == len(runs) - 1),
                                skip_group_check=True)
                    nc.vector.tensor_tensor(
                        out=vstage[:, b0 * R2:(b0 + nblk) * R2],
                        in0=ps2[:, :nblk * R2],
                        in1=sm2[:, :nblk * R2],
                        op=mybir.AluOpType.add)

                # ------------- final: normalize + log-softmax ----------------
                den2 = pcb.tile([P, NB], f32, tag="den2")
                nc.vector.tensor_scalar_max(out=den2[:],
                                            in0=_sub(vstage[:], CLS, [[R2, NB]]),
                                            scalar1=1e-20)
                rd2 = pcb.tile([P, NB], f32, tag="rd2")
                nc.vector.reciprocal(out=rd2[:], in_=den2[:])
                v = pcb.tile([P, NB * CLS], f32, tag="v")
                nc.vector.tensor_tensor(
                    out=v[:].rearrange("p (b k) -> p b k", b=NB),
                    in0=_sub(vstage[:], 0, [[R2, NB], [1, CLS]]),
                    in1=_sub(rd2[:], 0, [[1, NB], [0, CLS]]),
                    op=mybir.AluOpType.mult)
                nc.vector.tensor_tensor(
                    out=v[:].rearrange("p (b k) -> p b k", b=NB),
                    in0=v[:].rearrange("p (b k) -> p b k", b=NB),
                    in1=_sub(b2s[:], 0, [[0, NB], [1, CLS]]),
                    op=mybir.AluOpType.add)
                mx = pcb.tile([P, NB], f32, tag="mx")
                nc.vector.tensor_tensor(out=mx[:],
                                        in0=_sub(v[:], 0, [[CLS, NB]]),
                                        in1=_sub(v[:], 1, [[CLS, NB]]),
                                        op=mybir.AluOpType.max)
                u = pcb.tile([P, NB * CLS], f32, tag="u")
                nc.vector.tensor_tensor(
                    out=u[:].rearrange("p (b k) -> p b k", b=NB),
                    in0=v[:].rearrange("p (b k) -> p b k", b=NB),
                    in1=_sub(mx[:], 0, [[1, NB], [0, CLS]]),
                    op=mybir.AluOpType.subtract)
                nc.scalar.activation(out=u[:], in_=u[:],
                                     func=mybir.ActivationFunctionType.Exp)
                sm = pcb.tile([P, NB], f32, tag="sm")
                nc.vector.tensor_tensor(out=sm[:],
                                        in0=_sub(u[:], 0, [[CLS, NB]]),
                                        in1=_sub(u[:], 1, [[CLS, NB]]),
                                        op=mybir.AluOpType.add)
                ls = pcb.tile([P, NB], f32, tag="ls")
                nc.scalar.activation(out=ls[:], in_=sm[:],
                                     func=mybir.ActivationFunctionType.Ln)
                nc.vector.tensor_tensor(out=ls[:], in0=ls[:], in1=mx[:],
                                        op=mybir.AluOpType.add)
                res = pcb.tile([P, NB * CLS], f32, tag="res")
                nc.vector.tensor_tensor(
                    out=res[:].rearrange("p (b k) -> p b k", b=NB),
                    in0=v[:].rearrange("p (b k) -> p b k", b=NB),
                    in1=_sub(ls[:], 0, [[1, NB], [0, CLS]]),
                    op=mybir.AluOpType.subtract)
                nc.sync.dma_start(
                    out=bass.AP(out_d, 0, [[NB * CLS, P], [1, NB * CLS]]),
                    in_=res[:])
    nc.finalize()
    return nc


def install_ntff_hook(so_path="/opt/axon/libaxon_pjrt.so"):
    import types
    import ctypes
    import contextlib
    import antenv

    if getattr(antenv, "axon_hooks", None) is not None:
        return
    lib = ctypes.CDLL(so_path)
    if not hasattr(lib, "axon_start_nrt_profile"):
        return
    lib.axon_start_nrt_profile.argtypes = [ctypes.POINTER(ctypes.c_int64),
                                           ctypes.c_size_t]
    lib.axon_start_nrt_profile.restype = ctypes.c_int64
    lib.axon_stop_nrt_profile.argtypes = [ctypes.c_char_p]
    lib.axon_stop_nrt_profile.restype = ctypes.c_int64

    @contextlib.contextmanager
    def _hook(output_dir, device_ids):
        import jax
        jax.devices()
        if device_ids:
            ids = (ctypes.c_int64 * len(device_ids))(*device_ids)
            rc = lib.axon_start_nrt_profile(ids, len(device_ids))
        else:
            rc = lib.axon_start_nrt_profile(None, 0)
        if rc != 0:
            raise RuntimeError(f"axon_start_nrt_profile rc={rc}")
        try:
            yield
        finally:
            n = lib.axon_stop_nrt_profile(str(output_dir).encode())
            print(f"ntff profile: {n} file(s) written to {output_dir}")

    mod = types.ModuleType("antenv.axon_hooks")
    _reg = [_hook]
    mod.set_axon_ntff_profile_hook = lambda h: _reg.__setitem__(0, h)
    mod.get_axon_ntff_profile_hook = lambda: _reg[0]
    sys.modules["antenv.axon_hooks"] = mod
    antenv.axon_hooks = mod


def run(inputs, cfg, trace=False, **kwargs):
    if trace:
        install_ntff_hook()
    in_maps, meta = prep(inputs, cfg)
    nc = build(meta)
    res = bass_utils.run_bass_kernel_spmd(
        nc, in_maps, core_ids=list(range(cfg["NC"])), trace=trace, **kwargs)
    NPC, NB, N = meta["NPC"], meta["NB"], meta["N"]
    parts = []
    for c in range(cfg["NC"]):
        r = np.asarray(res.results[c]["out"])          # [NPC, 2], (p, b) order
        r = r.reshape(P, NB, cfg["CLS"]).transpose(1, 0, 2).reshape(NPC, cfg["CLS"])
        parts.append(r)
    out = np.concatenate(parts, axis=0)[:N]
    return out, res


# ----------------------------------------------------------------------------
# harness entry point
# ----------------------------------------------------------------------------

_CFG = dict(N=100000, F=165, H=4, C=64, CLS=2, NC=8, SBG=4)


def kernel(**inputs):
    """Full (unsharded) inputs -> full [N, 2] float32 log-softmax output.

    Shards edges by destination-node range across the 8 NeuronCores,
    compiles and runs the Bass/Tile kernel via run_bass_kernel_spmd,
    and reassembles the per-core output slices.
    """
    out, _ = run(inputs, _CFG, trace=False)
    return np.ascontiguousarray(out.astype(np.float32))


# revision 61
# speedup vs baseline: 1.0746x; 1.0746x over previous
"""GAT 2-layer message-passing network on 8 TRN2 NeuronCores (Bass/Tile).

v3: restructured around the v2 trace findings (phase A Sync-issue-bound,
phases B/C gather-DGE + small-op bound, 337us repack of tiny descriptors).

Strategy (dst-sharded, uniform NPC=12544 with tail pad nodes):
 - Core c owns nodes [c*12544, (c+1)*12544) (core 7 has 352 pad nodes) and
   all real (non-self-loop) edges into them. Self loops are handled
   analytically on-chip (diagonal add), NOT via gather slots -- this cuts
   slot padding sharply.
 - Each core computes h only for its OWN nodes (mini-pass, 7 slab loads /
   stores with 128 large descriptors each, local pi rows p*98+b), then one
   AllGather replicates hloc into the Shared table htabS [100352, 768B] at
   rows pi(n) = c*12544 + (nl%128)*98 + nl//128. The layer-2 table h2tab64
   [100352, 256B] uses the same pi, so BOTH edge phases share one slot
   geometry, one gather-chunk function q = src//25088 (int16-safe indices),
   and one index array; only the table/row size differ.
 - Slots: per superblock (4 dst blocks) x chunk runs, tiles of 128 slots may
   span blocks; boundary tiles get one one-hot column-set per touched block
   (dloc sentinel 255 masks foreign slots), so padding is per-(sb,q) only.
 - Per sb: gather 768B rows; a_dst per slot via oT one-hot matmuls from
   SBUF-resident slocS; ex=exp(lrelu(asrc+adst)); msg in-place; per-block
   PSUM aggregation via oh one-hot matmuls; self-loop contribution added as
   vector ops from an hloc row load; batched (per-sb) normalize + bias +
   relu + W2 matmul; h2 rows staged and stored in pi_C layout.
 - AllGather h2loc64 [12544,64]f32 -> Shared h2tab64 [100352,64].
 - Phase C: same slots, 256B-row gathers, batched epilogue into vstage;
   single final log-softmax over all blocks and one pi-ordered output store.
"""
import sys

if "/opt/trn_rl_repo" not in sys.path:
    sys.path.insert(0, "/opt/trn_rl_repo")

import math
import numpy as np
import ml_dtypes

import concourse.bass as bass
import concourse.bacc as bacc
import concourse.mybir as mybir
import concourse.tile as tile
from concourse import bass_utils

P = 128
NEG = 0.2
NCHUNK = 4
NQUEUE = 4
SLAB = 14                 # phase-A tiles per slab (14 | 196)
MAXT = 7                  # tiles per dma_gather call (1024-desc rings)
DMA_SCRATCH = 16384       # SWDGE carveout bytes/partition (1024 descs/queue)

# Tile's DMASW sem-lane assignment round-robins over all Pool DMAs, which
# breaks the per-lane FIFO assumption when SWDGE DMAs run on multiple queues
# (out-of-order completion across queues under one counting sem). Patch the
# lane choice to lane == queue_num: per-lane FIFO again holds (each HW ring
# drains in order), and queues get independent lanes.
from concourse import tile_sem_assignment as _tsa  # noqa: E402

if not getattr(_tsa.TileClockTick, "_qaware_patched", False):
    _orig_assign_tick = _tsa.TileClockTick._assign_tick

    def _qaware_assign_tick(self, inst):
        if (isinstance(inst, _tsa.DMAInst)
                and inst.engine == mybir.EngineType.Pool):
            self.next_sw_dma_idx = getattr(inst, "queue_num", 0) or 0
        return _orig_assign_tick(self, inst)

    _tsa.TileClockTick._assign_tick = _qaware_assign_tick
    _tsa.TileClockTick._qaware_patched = True


def _wrap16(flat):
    """[n] -> [128, n//16] wrapped in 16 partitions, replicated x8."""
    w = flat.reshape(-1, 16).T
    return np.tile(w, (8, 1))


# ----------------------------------------------------------------------------
# host-side data prep
# ----------------------------------------------------------------------------

def prep(inputs, cfg):
    N, F, H, C, CLS, NC = cfg["N"], cfg["F"], cfg["H"], cfg["C"], cfg["CLS"], cfg["NC"]
    SBG = cfg.get("SBG", 4)
    x = np.asarray(inputs["x"], np.float32)
    ei = np.asarray(inputs["edge_index"])
    W1 = np.asarray(inputs["W1"], np.float32)
    as1 = np.asarray(inputs["att_src1"], np.float32)
    ad1 = np.asarray(inputs["att_dst1"], np.float32)
    b1 = np.asarray(inputs["b1"], np.float32)
    W2 = np.asarray(inputs["W2"], np.float32)
    as2 = np.asarray(inputs["att_src2"], np.float32)
    ad2 = np.asarray(inputs["att_dst2"], np.float32)
    b2 = np.asarray(inputs["b2"], np.float32)

    HC = H * C                        # 256
    R1 = HC + H                       # gathered live row: [h | asrc]
    RG = 128 * math.ceil((R1 + H) / 128)  # 384 bf16 elems (768B rows)
    NPC, NB = 12544, 98
    NT = 784
    Np = NT * P                       # 100352
    CHB = Np // NCHUNK                # 25088 = 196*128 = 2*NPC
    TPC = CHB // P                    # 196 tiles per chunk
    RL2 = 64                          # f32 row elems for L2 table (256B)

    # ---- weights / constants -------------------------------------------------
    W1r = W1.reshape(F, H, C)
    Wsrc = np.einsum("fhc,hc->fh", W1r, as1)
    Wdst = np.einsum("fhc,hc->fh", W1r, ad1)
    W1aug = np.concatenate([W1, Wsrc, Wdst], axis=1)          # [F, 264]
    Wsrc2 = W2 @ as2.reshape(CLS, 1)
    Wdst2 = W2 @ ad2.reshape(CLS, 1)
    W2aug = np.concatenate([W2, Wsrc2, Wdst2], axis=1)        # [HC, 4]

    bf16 = ml_dtypes.bfloat16
    xT = np.zeros((F, Np), dtype=bf16)
    xT[:, :N] = x.T.astype(bf16)
    W1aug_b = W1aug.astype(bf16)
    W2aug_b = W2aug.astype(bf16)
    b1rep = np.tile(b1[None, :], (P, 1)).astype(bf16)
    b2rep = np.tile(b2[None, :], (P, 1)).astype(np.float32)
    iota = np.tile(np.arange(P, dtype=np.float32)[None, :], (P, 1)).astype(bf16)
    ident = np.eye(P, dtype=bf16)

    # ---- edges (real only; self loops handled on-chip) -----------------------
    src_all = np.asarray(ei[0], np.int64)
    dst_all = np.asarray(ei[1], np.int64)
    order = np.argsort(dst_all, kind="stable")
    src_s = src_all[order]
    dst_s = dst_all[order]
    q_s = src_s // CHB                                        # phase chunk

    # superblocks of dst blocks
    sblocks = [list(range(i, min(i + SBG, NB))) for i in range(0, NB, SBG)]

    # per-core, per-(sb, q, block) counts
    nsb = len(sblocks)
    cnt = np.zeros((NC, nsb, NCHUNK, SBG), np.int64)
    for c in range(NC):
        for si, blist in enumerate(sblocks):
            for bi, b in enumerate(blist):
                lo = c * NPC + b * P
                lo_i, hi_i = np.searchsorted(dst_s, lo), np.searchsorted(dst_s, lo + P)
                qs = q_s[lo_i:hi_i]
                for q in range(NCHUNK):
                    cnt[c, si, q, bi] = (qs == q).sum()
    cnt_sq = cnt.sum(axis=3)                                  # [NC, nsb, q]
    Trun = np.ceil(cnt_sq / P).astype(np.int64).max(axis=0)   # [nsb, q]
    Trun = np.maximum(Trun, 1)

    # slot layout + instance structure (global, core-agnostic)
    sb_meta = []
    tile_base = 0
    oh_base = 0
    for si, blist in enumerate(sblocks):
        segs = []               # per q: (tile_base_global, T)
        sb_tb = tile_base
        sb_ohb = oh_base
        pad_groups = []         # per tile_rel: list of (ohcol_rel, bi)
        agg = {bi: [] for bi in range(len(blist))}   # bi -> [(tile_rel, ohcol_rel)]
        inst_desc = []          # (tile_rel, bi) in oh column order
        for q in range(NCHUNK):
            T = int(Trun[si, q])
            segs.append((tile_base, T))
            # instance structure: union over cores of block spans
            # block bi span in run for core c: [off[c][bi], off[c][bi+1])
            offs = np.zeros((NC, len(blist) + 1), np.int64)
            for c in range(NC):
                offs[c, 1:] = np.cumsum(cnt[c, si, q, :len(blist)])
            for t in range(T):
                t_rel_global = tile_base - sb_tb + t
                s0, s1 = t * P, (t + 1) * P
                for bi in range(len(blist)):
                    hit = False
                    for c in range(NC):
                        if offs[c, bi] < s1 and offs[c, bi + 1] > s0:
                            hit = True
                            break
                    if hit:
                        inst_desc.append((t_rel_global, bi))
            tile_base += T
        S = tile_base - sb_tb
        Sx = len(inst_desc)
        oh_base += Sx
        pad_groups = [[] for _ in range(S)]
        for ohc, (t_rel, bi) in enumerate(inst_desc):
            pad_groups[t_rel].append((ohc, bi))
            agg[bi].append((t_rel, ohc))
        sb_meta.append(dict(base=sb_tb, S=S, ohbase=sb_ohb, Sx=Sx, segs=segs,
                            blocks=blist, b0=blist[0], inst=inst_desc,
                            pad_groups=pad_groups, agg=agg))
    Tsum = tile_base
    SxT = oh_base

    # per-core slot-value arrays
    ihC_w = np.zeros((NC, P, Tsum * 8), np.int16)
    dlx2d = np.zeros((NC, P, SxT), bf16)
    dlxT = np.zeros((NC, 1, SxT * P), bf16)
    oTh = np.zeros((NC, P, SxT * P), bf16)
    for c in range(NC):
        ihC = np.zeros(Tsum * P, np.int16)
        dlx = np.full(SxT * P, 255.0, np.float32)
        for si, blist in enumerate(sblocks):
            sb = sb_meta[si]
            for q in range(NCHUNK):
                tb, T = sb["segs"][q]
                # this core's edges for (sb, q), dst-sorted
                lo = c * NPC + blist[0] * P
                hi = c * NPC + blist[-1] * P + P
                lo_i, hi_i = np.searchsorted(dst_s, lo), np.searchsorted(dst_s, hi)
                m = q_s[lo_i:hi_i] == q
                es = src_s[lo_i:hi_i][m]
                ed = dst_s[lo_i:hi_i][m]
                n = len(es)
                assert n <= T * P, (n, T * P)
                s0 = tb * P
                # row idx within chunk q (pi_C layout, used by both phases)
                cs = es // NPC
                loc = es % NPC
                ihC[s0:s0 + n] = ((cs % 2) * NPC + (loc % P) * NB
                                  + loc // P).astype(np.int16)
                # dloc per instance column
                blk = (ed - c * NPC) // P - blist[0]          # bi of each edge
                dloc = ed - (c * NPC + (blist[0] + blk) * P)  # 0..127
                for ohc, (t_rel, bi) in enumerate(sb["inst"]):
                    pass
                # fill instance columns for this (sb, q)
                for t in range(T):
                    t_rel = tb - sb["base"] + t
                    e0, e1 = t * P, min((t + 1) * P, n)
                    if e0 >= n:
                        continue
                    for (ohc, bi) in sb["pad_groups"][t_rel]:
                        col0 = (sb["ohbase"] + ohc) * P
                        idx = np.arange(e0, e1)
                        sel = blk[idx] == bi
                        lanes = idx - t * P
                        vals = np.full(len(idx), 255.0, np.float32)
                        vals[sel] = dloc[idx[sel]]
                        dlx[col0 + lanes] = vals
        ihC_w[c] = _wrap16(ihC)
        dlx2d[c] = dlx.reshape(SxT, P).T.astype(bf16)
        dlxT[c, 0] = dlx.astype(bf16)
        oTh[c] = (np.arange(P, dtype=np.float32)[:, None]
                  == dlx[None, :]).astype(bf16)

    shared = {
        "W1aug": W1aug_b, "W2aug": W2aug_b, "b1rep": b1rep,
        "b2rep": b2rep, "iota": iota, "ident": ident,
        "iotac": np.arange(P, dtype=np.float32).reshape(P, 1),
        "onesk": np.ones((1, P), bf16),
    }
    in_maps = []
    for c in range(NC):
        m = dict(shared)
        m["xTloc"] = np.ascontiguousarray(xT[:, c * NPC:(c + 1) * NPC])
        m["ihsrcC"] = ihC_w[c]
        m["dlx2d"] = dlx2d[c]
        m["dlxT"] = dlxT[c]
        m["oTh"] = oTh[c]
        in_maps.append(m)

    meta = dict(cfg, R1=R1, RG=RG, HC=HC, NPC=NPC, NB=NB, NT=NT, Np=Np,
                CHB=CHB, TPC=TPC, RL2=RL2, Tsum=Tsum, SxT=SxT,
                sb_meta=sb_meta, SBG=SBG)
    return in_maps, meta


# ----------------------------------------------------------------------------
# device program
# ----------------------------------------------------------------------------

def _sub(ap, elem_off, dims):
    return bass.AP(ap.tensor, ap.offset + elem_off, [ap.ap[0], *list(dims)])


def build(meta, nc=None):
    N, F, H, C, CLS = meta["N"], meta["F"], meta["H"], meta["C"], meta["CLS"]
    NC, R1, RG, HC = meta["NC"], meta["R1"], meta["RG"], meta["HC"]
    NPC, NB, NT, Np = meta["NPC"], meta["NB"], meta["NT"], meta["Np"]
    CHB, TPC, RL2 = meta["CHB"], meta["TPC"], meta["RL2"]
    Tsum, SxT = meta["Tsum"], meta["SxT"]
    sb_meta = meta["sb_meta"]
    SBG = meta["SBG"]
    R2 = 4

    f32, bf16, i16 = mybir.dt.float32, mybir.dt.bfloat16, mybir.dt.int16

    if nc is None:
        nc = bacc.Bacc("TRN2", target_bir_lowering=False, debug=False,
                       num_devices=NC, num_swdge_queues=NQUEUE,
                       dynamic_dma_scratch_size=DMA_SCRATCH)

    qrr = [0]

    def gather_split(out_tile, rel, segT, elem, table, ix_tile):
        """Split a segment gather into <=MAXT-tile calls, round-robin queues."""
        done = 0
        while done < segT:
            tt = min(MAXT, segT - done)
            r = rel + done
            nc.gpsimd.dma_gather(
                bass.AP(out_tile[:].tensor, out_tile[:].offset + r * elem,
                        [out_tile[:].ap[0], [elem, tt], [1, elem]]),
                table,
                ix_tile[:, r * 8:(r + tt) * 8],
                tt * P, tt * P, elem,
                queue_num=qrr[0] % NQUEUE,
            )
            qrr[0] += 1
            done += tt

    xTl_d = nc.dram_tensor("xTloc", [F, NPC], bf16, kind="ExternalInput")
    W1aug_d = nc.dram_tensor("W1aug", [F, R1 + H], bf16, kind="ExternalInput")
    W2aug_d = nc.dram_tensor("W2aug", [HC, R2], bf16, kind="ExternalInput")
    b1rep_d = nc.dram_tensor("b1rep", [P, HC], bf16, kind="ExternalInput")
    b2rep_d = nc.dram_tensor("b2rep", [P, CLS], f32, kind="ExternalInput")
    iota_d = nc.dram_tensor("iota", [P, P], bf16, kind="ExternalInput")
    ident_d = nc.dram_tensor("ident", [P, P], bf16, kind="ExternalInput")
    ihC_d = nc.dram_tensor("ihsrcC", [P, Tsum * 8], i16, kind="ExternalInput")
    dlx2d_d = nc.dram_tensor("dlx2d", [P, SxT], bf16, kind="ExternalInput")
    dlxT_d = nc.dram_tensor("dlxT", [1, SxT * P], bf16, kind="ExternalInput")
    oTh_d = nc.dram_tensor("oTh", [P, SxT * P], bf16, kind="ExternalInput")
    iotac_d = nc.dram_tensor("iotac", [P, 1], f32, kind="ExternalInput")
    onesk_d = nc.dram_tensor("onesk", [1, P], bf16, kind="ExternalInput")
    out_d = nc.dram_tensor("out", [NPC, CLS], f32, kind="ExternalOutput")

    hloc = nc.dram_tensor("hloc", [NPC, RG], bf16, kind="Internal")
    htabS = nc.dram_tensor("htabS", [Np, RG], bf16, kind="Internal",
                           addr_space="Shared")
    h2loc64 = nc.dram_tensor("h2loc64", [NPC, RL2], f32, kind="Internal")
    h2tab64 = nc.dram_tensor("h2tab64", [Np, RL2], f32, kind="Internal",
                             addr_space="Shared")

    FA = min(P, F)
    FB = F - FA

    with tile.TileContext(nc) as tc:
        with tc.tile_pool(name="const", bufs=1) as cp:
            w1a = cp.tile([FA, R1 + H], bf16)
            nc.sync.dma_start(out=w1a[:], in_=W1aug_d[0:FA, :])
            w1b = cp.tile([FB, R1 + H], bf16)
            nc.sync.dma_start(out=w1b[:], in_=W1aug_d[FA:F, :])
            w2a = cp.tile([P, R2], bf16)
            nc.sync.dma_start(out=w2a[:], in_=W2aug_d[0:P, :])
            w2b = cp.tile([P, R2], bf16)
            nc.sync.dma_start(out=w2b[:], in_=W2aug_d[P:HC, :])
            b1s = cp.tile([P, HC], bf16)
            nc.sync.dma_start(out=b1s[:], in_=b1rep_d[:, :])
            b2s = cp.tile([P, CLS], f32)
            nc.sync.dma_start(out=b2s[:], in_=b2rep_d[:, :])
            iot = cp.tile([P, P], bf16)
            nc.sync.dma_start(out=iot[:], in_=iota_d[:, :])
            idn = cp.tile([P, P], bf16)
            nc.sync.dma_start(out=idn[:], in_=ident_d[:, :])
            dlc = cp.tile([P, SxT], bf16)
            nc.sync.dma_start(out=dlc[:], in_=dlx2d_d[:, :])
            iotc = cp.tile([P, 1], f32)
            nc.sync.dma_start(out=iotc[:], in_=iotac_d[:, :])
            onek = cp.tile([1, P], bf16)
            nc.sync.dma_start(out=onek[:], in_=onesk_d[:, :])
            # persistent SBUF state
            slocS = cp.tile([P, NB * H], bf16)       # a_dst of own nodes
            h2self = cp.tile([P, NB * R2], f32)      # own h2 rows
            vstage = cp.tile([P, NB * R2], f32)      # phase-C accumulators

            # ---------------- mini-pass: h table (own slice) -----------------
            with tc.tile_pool(name="pm", bufs=2) as pm, \
                 tc.tile_pool(name="psm", bufs=4, space="PSUM") as psm:
                for s in range(NPC // (SLAB * P)):          # 7 slabs
                    c0 = s * SLAB * P
                    xa = pm.tile([FA, SLAB * P], bf16, tag="xa")
                    nc.sync.dma_start(out=xa[:], in_=xTl_d[0:FA, c0:c0 + SLAB * P])
                    xb = pm.tile([FB, SLAB * P], bf16, tag="xb")
                    nc.sync.dma_start(out=xb[:], in_=xTl_d[FA:F, c0:c0 + SLAB * P])
                    stg = pm.tile([P, SLAB * RG], bf16, tag="stg")
                    for t in range(SLAB):
                        ph = psm.tile([P, R1 + H], f32, tag="ph")
                        nc.tensor.matmul(out=ph[:], lhsT=xa[:, t * P:(t + 1) * P],
                                         rhs=w1a[:], start=True, stop=False)
                        nc.tensor.matmul(out=ph[:], lhsT=xb[:, t * P:(t + 1) * P],
                                         rhs=w1b[:], start=False, stop=True)
                        nc.vector.tensor_copy(out=stg[:, t * RG:t * RG + R1],
                                              in_=ph[:, :R1])
                        nc.vector.tensor_copy(
                            out=slocS[:, (s * SLAB + t) * H:(s * SLAB + t + 1) * H],
                            in_=ph[:, R1:R1 + H])
                    nc.sync.dma_start(
                        out=bass.AP(hloc, s * SLAB * RG,
                                    [[NB * RG, P], [RG, SLAB], [1, RG]]),
                        in_=stg[:])

            # ---------------- AllGather h table ------------------------------
            nc.gpsimd.collective_compute(
                "AllGather", mybir.AluOpType.bypass,
                replica_groups=[list(range(NC))],
                ins=[hloc[:, :]], outs=[htabS[:, :]])

            # ---------------- Phase B: L1 edge pass --------------------------
            with tc.tile_pool(name="pbg", bufs=3) as pbg, \
                 tc.tile_pool(name="pbo", bufs=2) as pbo, \
                 tc.tile_pool(name="pbb", bufs=3) as pbb, \
                 tc.tile_pool(name="psb", bufs=4, space="PSUM") as psb, \
                 tc.tile_pool(name="psp", bufs=2, space="PSUM") as psp, \
                 tc.tile_pool(name="pst", bufs=1, space="PSUM") as pst, \
                 tc.tile_pool(name="psh", bufs=1, space="PSUM") as psh:
                for sb in sb_meta:
                    base, S, Sx, ohb = sb["base"], sb["S"], sb["Sx"], sb["ohbase"]
                    blist, b0 = sb["blocks"], sb["b0"]
                    nblk = len(blist)
                    g = pbg.tile([P, S * RG], bf16, tag="g")
                    ixs = pbg.tile([P, S * 8], i16, tag="ixs")
                    nc.sync.dma_start(out=ixs[:],
                                      in_=ihC_d[:, base * 8:(base + S) * 8])
                    for q in range(NCHUNK):
                        tb, T = sb["segs"][q]
                        gather_split(g, tb - base, T, RG,
                                     htabS[q * CHB:(q + 1) * CHB, :], ixs)
                    selfh = pbg.tile([P, SBG * RG], bf16, tag="selfh")
                    nc.sync.dma_start(
                        out=selfh[:, :nblk * RG],
                        in_=bass.AP(hloc, b0 * RG,
                                    [[NB * RG, P], [1, nblk * RG]]))
                    # oT: [dst_local, inst_col] one-hot (host-precomputed);
                    # split load so early pad matmuls start at half-load
                    oT = pbo.tile([P, Sx * P], bf16, tag="oT")
                    h1 = (Sx // 2) * P
                    nc.sync.dma_start(out=oT[:, :h1],
                                      in_=oTh_d[:, ohb * P:ohb * P + h1])
                    nc.sync.dma_start(out=oT[:, h1:Sx * P],
                                      in_=oTh_d[:, ohb * P + h1:(ohb + Sx) * P])
                    # per-slot a_dst via oT matmuls -> PSUM [P, S*H]
                    pad = psp.tile([P, S * H], f32, tag="pad")
                    for t in range(S):
                        grp = sb["pad_groups"][t]
                        for gi, (ohc, bi) in enumerate(grp):
                            nc.tensor.matmul(
                                out=pad[:, t * H:(t + 1) * H],
                                lhsT=oT[:, ohc * P:(ohc + 1) * P],
                                rhs=slocS[:, (b0 + bi) * H:(b0 + bi + 1) * H],
                                start=(gi == 0), stop=(gi == len(grp) - 1),
                                skip_group_check=True)
                    # ex = exp(lrelu(asrc+adst))  [P, S*H] f32
                    ex = pbb.tile([P, S * H], f32, tag="ex")
                    nc.vector.tensor_tensor(
                        out=ex[:].rearrange("p (t h) -> p t h", t=S),
                        in0=_sub(g[:], HC, [[RG, S], [1, H]]),
                        in1=pad[:].rearrange("p (t h) -> p t h", t=S),
                        op=mybir.AluOpType.add)
                    tmp = pbb.tile([P, S * H], f32, tag="tmp")
                    nc.vector.tensor_scalar_mul(out=tmp[:], in0=ex[:], scalar1=NEG)
                    nc.vector.tensor_tensor(out=ex[:], in0=ex[:], in1=tmp[:],
                                            op=mybir.AluOpType.max)
                    nc.scalar.activation(out=ex[:], in_=ex[:],
                                         func=mybir.ActivationFunctionType.Exp)
                    exb = pbb.tile([P, S * H], bf16, tag="exb")
                    nc.vector.tensor_copy(out=exb[:], in_=ex[:])
                    # msg in-place: cols 0:HC *= ex ; cols HC:HC+H = ex
                    nc.vector.tensor_tensor(
                        out=_sub(g[:], 0, [[RG, S], [C, H], [1, C]]),
                        in0=_sub(g[:], 0, [[RG, S], [C, H], [1, C]]),
                        in1=_sub(exb[:], 0, [[H, S], [1, H], [0, C]]),
                        op=mybir.AluOpType.mult)
                    nc.vector.tensor_copy(
                        out=_sub(g[:], HC, [[RG, S], [1, H]]),
                        in_=exb[:].rearrange("p (t h) -> p t h", t=S))
                    # oh: [slot, inst_col] one-hot
                    oh = pbo.tile([P, Sx * P], bf16, tag="oh")
                    nc.vector.tensor_tensor(
                        out=oh[:].rearrange("p (t q) -> p t q", t=Sx),
                        in0=_sub(dlc[:], ohb, [[1, Sx], [0, P]]),
                        in1=_sub(iot[:], 0, [[0, Sx], [1, P]]),
                        op=mybir.AluOpType.is_equal)
                    # self-loop messages (batched over blocks)
                    lS = pbb.tile([P, SBG * H], f32, tag="lS")
                    nc.vector.tensor_tensor(
                        out=lS[:, :nblk * H].rearrange("p (b h) -> p b h", b=nblk),
                        in0=_sub(selfh[:], HC, [[RG, nblk], [1, H]]),
                        in1=_sub(slocS[:], b0 * H, [[H, nblk], [1, H]]),
                        op=mybir.AluOpType.add)
                    tS = pbb.tile([P, SBG * H], f32, tag="tS")
                    nc.vector.tensor_scalar_mul(out=tS[:], in0=lS[:], scalar1=NEG)
                    nc.vector.tensor_tensor(out=lS[:], in0=lS[:], in1=tS[:],
                                            op=mybir.AluOpType.max)
                    nc.scalar.activation(out=lS[:], in_=lS[:],
                                         func=mybir.ActivationFunctionType.Exp)
                    selfm = pbb.tile([P, SBG * (R1 + 4)], bf16, tag="selfm")
                    RS = R1 + 4
                    nc.vector.tensor_tensor(
                        out=_sub(selfm[:], 0, [[RS, nblk], [C, H], [1, C]]),
                        in0=_sub(selfh[:], 0, [[RG, nblk], [C, H], [1, C]]),
                        in1=_sub(lS[:], 0, [[H, nblk], [1, H], [0, C]]),
                        op=mybir.AluOpType.mult)
                    nc.vector.tensor_copy(
                        out=_sub(selfm[:], HC, [[RS, nblk], [1, H]]),
                        in_=lS[:, :nblk * H].rearrange("p (b h) -> p b h", b=nblk))
                    # per-block aggregation + batched epilogue
                    psoS = pbb.tile([P, SBG * RS], f32, tag="psoS")
                    for bi in range(nblk):
                        runs = sb["agg"][bi]
                        pso = psb.tile([P, R1], f32, tag="pso")
                        for ri, (t_rel, ohc) in enumerate(runs):
                            nc.tensor.matmul(
                                out=pso[:],
                                lhsT=oh[:, ohc * P:(ohc + 1) * P],
                                rhs=g[:, t_rel * RG:t_rel * RG + R1],
                                start=(ri == 0), stop=(ri == len(runs) - 1))
                        nc.vector.tensor_tensor(
                            out=psoS[:, bi * RS:bi * RS + R1],
                            in0=pso[:],
                            in1=selfm[:, bi * RS:bi * RS + R1],
                            op=mybir.AluOpType.add)
                    den = pbb.tile([P, SBG * H], f32, tag="den")
                    nc.vector.tensor_scalar_max(
                        out=den[:, :nblk * H].rearrange("p (b h) -> p b h", b=nblk),
                        in0=_sub(psoS[:], HC, [[RS, nblk], [1, H]]),
                        scalar1=1e-20)
                    rde = pbb.tile([P, SBG * H], f32, tag="rde")
                    nc.vector.reciprocal(out=rde[:], in_=den[:])
                    o1 = pbb.tile([P, SBG * HC], bf16, tag="o1")
                    nc.vector.tensor_tensor(
                        out=o1[:].rearrange("p (b h c) -> p b h c", b=SBG, h=H),
                        in0=_sub(psoS[:], 0, [[RS, SBG], [C, H], [1, C]]),
                        in1=_sub(rde[:], 0, [[H, SBG], [1, H], [0, C]]),
                        op=mybir.AluOpType.mult)
                    nc.vector.tensor_tensor(
                        out=o1[:].rearrange("p (b k) -> p b k", b=SBG),
                        in0=o1[:].rearrange("p (b k) -> p b k", b=SBG),
                        in1=_sub(b1s[:], 0, [[0, SBG], [1, HC]]),
                        op=mybir.AluOpType.add)
                    nc.scalar.activation(out=o1[:], in_=o1[:],
                                         func=mybir.ActivationFunctionType.Relu)
                    # h2 = relu(o1) @ W2aug via PE transposes
                    ptr = pst.tile([P, 2 * SBG * P], bf16, tag="ptr")
                    for bi in range(nblk):
                        for k in range(2):
                            nc.tensor.transpose(
                                out=ptr[:, (bi * 2 + k) * P:(bi * 2 + k + 1) * P],
                                in_=o1[:, bi * HC + k * P:bi * HC + (k + 1) * P],
                                identity=idn[:])
                    rT = pbb.tile([P, 2 * SBG * P], bf16, tag="rT")
                    nc.vector.tensor_copy(out=rT[:, :nblk * 2 * P],
                                          in_=ptr[:, :nblk * 2 * P])
                    ph2 = psh.tile([P, SBG * R2], f32, tag="ph2")
                    for bi in range(nblk):
                        nc.tensor.matmul(out=ph2[:, bi * R2:(bi + 1) * R2],
                                         lhsT=rT[:, bi * 2 * P:(bi * 2 + 1) * P],
                                         rhs=w2a[:], start=True, stop=False,
                                         skip_group_check=True)
                        nc.tensor.matmul(out=ph2[:, bi * R2:(bi + 1) * R2],
                                         lhsT=rT[:, (bi * 2 + 1) * P:(bi * 2 + 2) * P],
                                         rhs=w2b[:], start=False, stop=True,
                                         skip_group_check=True)
                    nc.vector.tensor_copy(out=h2self[:, b0 * R2:(b0 + nblk) * R2],
                                          in_=ph2[:, :nblk * R2])
                    h2st = pbb.tile([P, SBG * RL2], f32, tag="h2st")
                    nc.vector.tensor_copy(
                        out=_sub(h2st[:], 0, [[RL2, nblk], [1, R2]]),
                        in_=ph2[:, :nblk * R2].rearrange("p (b r) -> p b r", b=nblk))
                    nc.sync.dma_start(
                        out=bass.AP(h2loc64, b0 * RL2,
                                    [[NB * RL2, P], [1, nblk * RL2]]),
                        in_=h2st[:, :nblk * RL2])

            # ---------------- Phase C: L2 edge pass --------------------------
            # The collective barriers work issued AFTER it, so the first few
            # superblocks' AG-independent prep (ixs/dlT loads, oT build) is
            # issued BEFORE the AllGather to hide in its window.
            CPREP = 3
            with tc.tile_pool(name="pcg", bufs=4) as pcg, \
                 tc.tile_pool(name="pco", bufs=3) as pco, \
                 tc.tile_pool(name="pcb", bufs=2) as pcb, \
                 tc.tile_pool(name="psc", bufs=4, space="PSUM") as psc, \
                 tc.tile_pool(name="psp2", bufs=2, space="PSUM") as psp2, \
                 tc.tile_pool(name="psk2", bufs=2, space="PSUM") as psk2:
                cprep = []
                for sb in sb_meta[:CPREP]:
                    base, S, Sx, ohb = sb["base"], sb["S"], sb["Sx"], sb["ohbase"]
                    ixs = pcg.tile([P, S * 8], i16, tag="ixs2")
                    nc.sync.dma_start(out=ixs[:],
                                      in_=ihC_d[:, base * 8:(base + S) * 8])
                    dlT = pcg.tile([1, Sx * P], bf16, tag="dlT2")
                    nc.sync.dma_start(out=dlT[:],
                                      in_=dlxT_d[0:1, ohb * P:(ohb + Sx) * P])
                    oT = pco.tile([P, Sx * P], bf16, tag="oT2")
                    for st in range(0, Sx * P, 512):
                        w = min(512, Sx * P - st)
                        stp = psk2.tile([P, 512], f32, tag="stp2")
                        nc.tensor.matmul(out=stp[:, :w], lhsT=onek[:],
                                         rhs=dlT[0:1, st:st + w],
                                         start=True, stop=True)
                        nc.vector.tensor_tensor(
                            out=oT[:, st:st + w],
                            in0=iotc[:, 0:1].to_broadcast([P, w]),
                            in1=stp[:, :w],
                            op=mybir.AluOpType.is_equal)
                    cprep.append((ixs, oT))

                nc.gpsimd.collective_compute(
                    "AllGather", mybir.AluOpType.bypass,
                    replica_groups=[list(range(NC))],
                    ins=[h2loc64[:, :]], outs=[h2tab64[:, :]])

                for si, sb in enumerate(sb_meta):
                    base, S, Sx, ohb = sb["base"], sb["S"], sb["Sx"], sb["ohbase"]
                    blist, b0 = sb["blocks"], sb["b0"]
                    nblk = len(blist)
                    g2 = pcg.tile([P, S * RL2], f32, tag="g2")
                    if si < CPREP:
                        ixs, oT = cprep[si]
                    else:
                        ixs = pcg.tile([P, S * 8], i16, tag="ixs2")
                        nc.sync.dma_start(out=ixs[:],
                                          in_=ihC_d[:, base * 8:(base + S) * 8])
                    for q in range(NCHUNK):
                        tb, T = sb["segs"][q]
                        gather_split(g2, tb - base, T, RL2,
                                     h2tab64[q * CHB:(q + 1) * CHB, :], ixs)
                    if si >= CPREP:
                        dlT = pcg.tile([1, Sx * P], bf16, tag="dlT2")
                        nc.sync.dma_start(out=dlT[:],
                                          in_=dlxT_d[0:1, ohb * P:(ohb + Sx) * P])
                        oT = pco.tile([P, Sx * P], bf16, tag="oT2")
                        for st in range(0, Sx * P, 512):
                            w = min(512, Sx * P - st)
                            stp = psk2.tile([P, 512], f32, tag="stp2")
                            nc.tensor.matmul(out=stp[:, :w], lhsT=onek[:],
                                             rhs=dlT[0:1, st:st + w],
                                             start=True, stop=True)
                            nc.vector.tensor_tensor(
                                out=oT[:, st:st + w],
                                in0=iotc[:, 0:1].to_broadcast([P, w]),
                                in1=stp[:, :w],
                                op=mybir.AluOpType.is_equal)
                    adw2 = pcb.tile([P, SBG], bf16, tag="adw2")
                    nc.vector.tensor_copy(
                        out=adw2[:, :nblk],
                        in_=_sub(h2self[:], b0 * R2 + 3, [[R2, nblk]]))
                    pad2 = psp2.tile([P, S], f32, tag="pad2")
                    for t in range(S):
                        grp = sb["pad_groups"][t]
                        for gi, (ohc, bi) in enumerate(grp):
                            nc.tensor.matmul(
                                out=pad2[:, t:t + 1],
                                lhsT=oT[:, ohc * P:(ohc + 1) * P],
                                rhs=adw2[:, bi:bi + 1],
                                start=(gi == 0), stop=(gi == len(grp) - 1),
                                skip_group_check=True)
                    ex2 = pcb.tile([P, S], f32, tag="ex2")
                    nc.vector.tensor_tensor(
                        out=ex2[:],
                        in0=_sub(g2[:], CLS, [[RL2, S]]),
                        in1=pad2[:],
                        op=mybir.AluOpType.add)
                    tm2 = pcb.tile([P, S], f32, tag="tm2")
                    nc.vector.tensor_scalar_mul(out=tm2[:], in0=ex2[:], scalar1=NEG)
                    nc.vector.tensor_tensor(out=ex2[:], in0=ex2[:], in1=tm2[:],
                                            op=mybir.AluOpType.max)
                    nc.scalar.activation(out=ex2[:], in_=ex2[:],
                                         func=mybir.ActivationFunctionType.Exp)
                    m2 = pcb.tile([P, S * R2], bf16, tag="m2")
                    nc.vector.tensor_tensor(
                        out=_sub(m2[:], 0, [[R2, S], [1, CLS]]),
                        in0=_sub(g2[:], 0, [[RL2, S], [1, CLS]]),
                        in1=_sub(ex2[:], 0, [[1, S], [0, CLS]]),
                        op=mybir.AluOpType.mult)
                    nc.vector.tensor_copy(
                        out=_sub(m2[:], CLS, [[R2, S], [1, 2]]),
                        in_=_sub(ex2[:], 0, [[1, S], [0, 2]]))
                    oh = pco.tile([P, Sx * P], bf16, tag="oh2")
                    nc.vector.tensor_tensor(
                        out=oh[:].rearrange("p (t q) -> p t q", t=Sx),
                        in0=_sub(dlc[:], ohb, [[1, Sx], [0, P]]),
                        in1=_sub(iot[:], 0, [[0, Sx], [1, P]]),
                        op=mybir.AluOpType.is_equal)
                    # self-loop L2 messages
                    l2S = pcb.tile([P, SBG], f32, tag="l2S")
                    nc.vector.tensor_tensor(
                        out=l2S[:, :nblk],
                        in0=_sub(h2self[:], b0 * R2 + 2, [[R2, nblk]]),
                        in1=_sub(h2self[:], b0 * R2 + 3, [[R2, nblk]]),
                        op=mybir.AluOpType.add)
                    t2S = pcb.tile([P, SBG], f32, tag="t2S")
                    nc.vector.tensor_scalar_mul(out=t2S[:], in0=l2S[:], scalar1=NEG)
                    nc.vector.tensor_tensor(out=l2S[:], in0=l2S[:], in1=t2S[:],
                                            op=mybir.AluOpType.max)
                    nc.scalar.activation(out=l2S[:], in_=l2S[:],
                                         func=mybir.ActivationFunctionType.Exp)
                    sm2 = pcb.tile([P, SBG * R2], f32, tag="sm2")
                    nc.vector.tensor_tensor(
                        out=_sub(sm2[:], 0, [[R2, nblk], [1, CLS]]),
                        in0=_sub(h2self[:], b0 * R2, [[R2, nblk], [1, CLS]]),
                        in1=_sub(l2S[:], 0, [[1, nblk], [0, CLS]]),
                        op=mybir.AluOpType.mult)
                    nc.vector.tensor_copy(
                        out=_sub(sm2[:], CLS, [[R2, nblk], [1, 2]]),
                        in_=_sub(l2S[:], 0, [[1, nblk], [0, 2]]))
                    ps2 = psc.tile([P, SBG * R2], f32, tag="ps2")
                    for bi in range(nblk):
                        runs = sb["agg"][bi]
                        for ri, (t_rel, ohc) in enumerate(runs):
                            nc.tensor.matmul(
                                out=ps2[:, bi * R2:(bi + 1) * R2],
                                lhsT=oh[:, ohc * P:(ohc + 1) * P],
                                rhs=m2[:, t_rel * R2:(t_rel + 1) * R2],
                                start=(ri == 0), stop=(ri == len(runs) - 1),
                                skip_group_check=True)
                    nc.vector.tensor_tensor(
                        out=vstage[:, b0 * R2:(b0 + nblk) * R2],
                        in0=ps2[:, :nblk * R2],
                        in1=sm2[:, :nblk * R2],
                        op=mybir.AluOpType.add)

                # ------------- final: normalize + log-softmax ----------------
                den2 = pcb.tile([P, NB], f32, tag="den2")
                nc.vector.tensor_scalar_max(out=den2[:],
                                            in0=_sub(vstage[:], CLS, [[R2, NB]]),
                                            scalar1=1e-20)
                rd2 = pcb.tile([P, NB], f32, tag="rd2")
                nc.vector.reciprocal(out=rd2[:], in_=den2[:])
                v = pcb.tile([P, NB * CLS], f32, tag="v")
                nc.vector.tensor_tensor(
                    out=v[:].rearrange("p (b k) -> p b k", b=NB),
                    in0=_sub(vstage[:], 0, [[R2, NB], [1, CLS]]),
                    in1=_sub(rd2[:], 0, [[1, NB], [0, CLS]]),
                    op=mybir.AluOpType.mult)
                nc.vector.tensor_tensor(
                    out=v[:].rearrange("p (b k) -> p b k", b=NB),
                    in0=v[:].rearrange("p (b k) -> p b k", b=NB),
                    in1=_sub(b2s[:], 0, [[0, NB], [1, CLS]]),
                    op=mybir.AluOpType.add)
                mx = pcb.tile([P, NB], f32, tag="mx")
                nc.vector.tensor_tensor(out=mx[:],
                                        in0=_sub(v[:], 0, [[CLS, NB]]),
                                        in1=_sub(v[:], 1, [[CLS, NB]]),
                                        op=mybir.AluOpType.max)
                u = pcb.tile([P, NB * CLS], f32, tag="u")
                nc.vector.tensor_tensor(
                    out=u[:].rearrange("p (b k) -> p b k", b=NB),
                    in0=v[:].rearrange("p (b k) -> p b k", b=NB),
                    in1=_sub(mx[:], 0, [[1, NB], [0, CLS]]),
                    op=mybir.AluOpType.subtract)
                nc.scalar.activation(out=u[:], in_=u[:],
                                     func=mybir.ActivationFunctionType.Exp)
                sm = pcb.tile([P, NB], f32, tag="sm")
                nc.vector.tensor_tensor(out=sm[:],
                                        in0=_sub(u[:], 0, [[CLS, NB]]),
                                        in1=_sub(u[:], 1, [[CLS, NB]]),
                                        op=mybir.AluOpType.add)
                ls = pcb.tile([P, NB], f32, tag="ls")
                nc.scalar.activation(out=ls[:], in_=sm[:],
                                     func=mybir.ActivationFunctionType.Ln)
                nc.vector.tensor_tensor(out=ls[:], in0=ls[:], in1=mx[:],
                                        op=mybir.AluOpType.add)
                res = pcb.tile([P, NB * CLS], f32, tag="res")
                nc.vector.tensor_tensor(
                    out=res[:].rearrange("p (b k) -> p b k", b=NB),
                    in0=v[:].rearrange("p (b k) -> p b k", b=NB),
                    in1=_sub(ls[:], 0, [[1, NB], [0, CLS]]),
                    op=mybir.AluOpType.subtract)
                nc.sync.dma_start(
                    out=bass.AP(out_d, 0, [[NB * CLS, P], [1, NB * CLS]]),
                    in_=res[:])
    nc.finalize()
    return nc


def install_ntff_hook(so_path="/opt/axon/libaxon_pjrt.so"):
    import types
    import ctypes
    import contextlib
    import antenv

    if getattr(antenv, "axon_hooks", None) is not None:
        return
    lib = ctypes.CDLL(so_path)
    if not hasattr(lib, "axon_start_nrt_profile"):
        return
    lib.axon_start_nrt_profile.argtypes = [ctypes.POINTER(ctypes.c_int64),
                                           ctypes.c_size_t]
    lib.axon_start_nrt_profile.restype = ctypes.c_int64
    lib.axon_stop_nrt_profile.argtypes = [ctypes.c_char_p]
    lib.axon_stop_nrt_profile.restype = ctypes.c_int64

    @contextlib.contextmanager
    def _hook(output_dir, device_ids):
        import jax
        jax.devices()
        if device_ids:
            ids = (ctypes.c_int64 * len(device_ids))(*device_ids)
            rc = lib.axon_start_nrt_profile(ids, len(device_ids))
        else:
            rc = lib.axon_start_nrt_profile(None, 0)
        if rc != 0:
            raise RuntimeError(f"axon_start_nrt_profile rc={rc}")
        try:
            yield
        finally:
            n = lib.axon_stop_nrt_profile(str(output_dir).encode())
            print(f"ntff profile: {n} file(s) written to {output_dir}")

    mod = types.ModuleType("antenv.axon_hooks")
    _reg = [_hook]
    mod.set_axon_ntff_profile_hook = lambda h: _reg.__setitem__(0, h)
    mod.get_axon_ntff_profile_hook = lambda: _reg[0]
    sys.modules["antenv.axon_hooks"] = mod
    antenv.axon_hooks = mod


def run(inputs, cfg, trace=False, **kwargs):
    if trace:
        install_ntff_hook()
    in_maps, meta = prep(inputs, cfg)
    nc = build(meta)
    res = bass_utils.run_bass_kernel_spmd(
        nc, in_maps, core_ids=list(range(cfg["NC"])), trace=trace, **kwargs)
    NPC, NB, N = meta["NPC"], meta["NB"], meta["N"]
    parts = []
    for c in range(cfg["NC"]):
        r = np.asarray(res.results[c]["out"])          # [NPC, 2], (p, b) order
        r = r.reshape(P, NB, cfg["CLS"]).transpose(1, 0, 2).reshape(NPC, cfg["CLS"])
        parts.append(r)
    out = np.concatenate(parts, axis=0)[:N]
    return out, res


# ----------------------------------------------------------------------------
# harness entry point
# ----------------------------------------------------------------------------

_CFG = dict(N=100000, F=165, H=4, C=64, CLS=2, NC=8, SBG=4)


def kernel(**inputs):
    """Full (unsharded) inputs -> full [N, 2] float32 log-softmax output.

    Shards edges by destination-node range across the 8 NeuronCores,
    compiles and runs the Bass/Tile kernel via run_bass_kernel_spmd,
    and reassembles the per-core output slices.
    """
    out, _ = run(inputs, _CFG, trace=False)
    return np.ascontiguousarray(out.astype(np.float32))


# revision 63
# speedup vs baseline: 1.1202x; 1.0425x over previous
"""GAT 2-layer message-passing network on 8 TRN2 NeuronCores (Bass/Tile).

v3: restructured around the v2 trace findings (phase A Sync-issue-bound,
phases B/C gather-DGE + small-op bound, 337us repack of tiny descriptors).

Strategy (dst-sharded, uniform NPC=12544 with tail pad nodes):
 - Core c owns nodes [c*12544, (c+1)*12544) (core 7 has 352 pad nodes) and
   all real (non-self-loop) edges into them. Self loops are handled
   analytically on-chip (diagonal add), NOT via gather slots -- this cuts
   slot padding sharply.
 - Each core computes h only for its OWN nodes (mini-pass, 7 slab loads /
   stores with 128 large descriptors each, local pi rows p*98+b), then one
   AllGather replicates hloc into the Shared table htabS [100352, 768B] at
   rows pi(n) = c*12544 + (nl%128)*98 + nl//128. The layer-2 table h2tab64
   [100352, 256B] uses the same pi, so BOTH edge phases share one slot
   geometry, one gather-chunk function q = src//25088 (int16-safe indices),
   and one index array; only the table/row size differ.
 - Slots: per superblock (4 dst blocks) x chunk runs, tiles of 128 slots may
   span blocks; boundary tiles get one one-hot column-set per touched block
   (dloc sentinel 255 masks foreign slots), so padding is per-(sb,q) only.
 - Per sb: gather 768B rows; a_dst per slot via oT one-hot matmuls from
   SBUF-resident slocS; ex=exp(lrelu(asrc+adst)); msg in-place; per-block
   PSUM aggregation via oh one-hot matmuls; self-loop contribution added as
   vector ops from an hloc row load; batched (per-sb) normalize + bias +
   relu + W2 matmul; h2 rows staged and stored in pi_C layout.
 - AllGather h2loc64 [12544,64]f32 -> Shared h2tab64 [100352,64].
 - Phase C: same slots, 256B-row gathers, batched epilogue into vstage;
   single final log-softmax over all blocks and one pi-ordered output store.
"""
import sys

if "/opt/trn_rl_repo" not in sys.path:
    sys.path.insert(0, "/opt/trn_rl_repo")

import math
import numpy as np
import ml_dtypes

import concourse.bass as bass
import concourse.bacc as bacc
import concourse.mybir as mybir
import concourse.tile as tile
from concourse import bass_utils

P = 128
NEG = 0.2
NCHUNK = 4
NQUEUE = 4
SLAB = 14                 # phase-A tiles per slab (14 | 196)
MAXT = 5                  # tiles per call: near-even splits balance queues
DMA_SCRATCH = 16384       # SWDGE carveout bytes/partition (1024 descs/queue)

# Tile's DMASW sem-lane assignment round-robins over all Pool DMAs, which
# breaks the per-lane FIFO assumption when SWDGE DMAs run on multiple queues
# (out-of-order completion across queues under one counting sem). Patch the
# lane choice to lane == queue_num: per-lane FIFO again holds (each HW ring
# drains in order), and queues get independent lanes.
from concourse import tile_sem_assignment as _tsa  # noqa: E402

if not getattr(_tsa.TileClockTick, "_qaware_patched", False):
    _orig_assign_tick = _tsa.TileClockTick._assign_tick

    def _qaware_assign_tick(self, inst):
        if (isinstance(inst, _tsa.DMAInst)
                and inst.engine == mybir.EngineType.Pool):
            self.next_sw_dma_idx = getattr(inst, "queue_num", 0) or 0
        return _orig_assign_tick(self, inst)

    _tsa.TileClockTick._assign_tick = _qaware_assign_tick
    _tsa.TileClockTick._qaware_patched = True


def _wrap16(flat):
    """[n] -> [128, n//16] wrapped in 16 partitions, replicated x8."""
    w = flat.reshape(-1, 16).T
    return np.tile(w, (8, 1))


# ----------------------------------------------------------------------------
# host-side data prep
# ----------------------------------------------------------------------------

def prep(inputs, cfg):
    N, F, H, C, CLS, NC = cfg["N"], cfg["F"], cfg["H"], cfg["C"], cfg["CLS"], cfg["NC"]
    SBG = cfg.get("SBG", 4)
    x = np.asarray(inputs["x"], np.float32)
    ei = np.asarray(inputs["edge_index"])
    W1 = np.asarray(inputs["W1"], np.float32)
    as1 = np.asarray(inputs["att_src1"], np.float32)
    ad1 = np.asarray(inputs["att_dst1"], np.float32)
    b1 = np.asarray(inputs["b1"], np.float32)
    W2 = np.asarray(inputs["W2"], np.float32)
    as2 = np.asarray(inputs["att_src2"], np.float32)
    ad2 = np.asarray(inputs["att_dst2"], np.float32)
    b2 = np.asarray(inputs["b2"], np.float32)

    HC = H * C                        # 256
    R1 = HC + H                       # gathered live row: [h | asrc]
    RG = 128 * math.ceil((R1 + H) / 128)  # 384 bf16 elems (768B rows)
    NPC, NB = 12544, 98
    NT = 784
    Np = NT * P                       # 100352
    CHB = Np // NCHUNK                # 25088 = 196*128 = 2*NPC
    TPC = CHB // P                    # 196 tiles per chunk
    RL2 = 64                          # f32 row elems for L2 table (256B)

    # ---- weights / constants -------------------------------------------------
    W1r = W1.reshape(F, H, C)
    Wsrc = np.einsum("fhc,hc->fh", W1r, as1)
    Wdst = np.einsum("fhc,hc->fh", W1r, ad1)
    W1aug = np.concatenate([W1, Wsrc, Wdst], axis=1)          # [F, 264]
    Wsrc2 = W2 @ as2.reshape(CLS, 1)
    Wdst2 = W2 @ ad2.reshape(CLS, 1)
    W2aug = np.concatenate([W2, Wsrc2, Wdst2], axis=1)        # [HC, 4]

    bf16 = ml_dtypes.bfloat16
    xT = np.zeros((F, Np), dtype=bf16)
    xT[:, :N] = x.T.astype(bf16)
    W1aug_b = W1aug.astype(bf16)
    W2aug_b = W2aug.astype(bf16)
    b1rep = np.tile(b1[None, :], (P, 1)).astype(bf16)
    b2rep = np.tile(b2[None, :], (P, 1)).astype(np.float32)
    iota = np.tile(np.arange(P, dtype=np.float32)[None, :], (P, 1)).astype(bf16)
    ident = np.eye(P, dtype=bf16)

    # ---- edges (real only; self loops handled on-chip) -----------------------
    src_all = np.asarray(ei[0], np.int64)
    dst_all = np.asarray(ei[1], np.int64)
    order = np.argsort(dst_all, kind="stable")
    src_s = src_all[order]
    dst_s = dst_all[order]
    q_s = src_s // CHB                                        # phase chunk

    # superblocks of dst blocks
    sblocks = [list(range(i, min(i + SBG, NB))) for i in range(0, NB, SBG)]

    # per-core, per-(sb, q, block) counts
    nsb = len(sblocks)
    cnt = np.zeros((NC, nsb, NCHUNK, SBG), np.int64)
    for c in range(NC):
        for si, blist in enumerate(sblocks):
            for bi, b in enumerate(blist):
                lo = c * NPC + b * P
                lo_i, hi_i = np.searchsorted(dst_s, lo), np.searchsorted(dst_s, lo + P)
                qs = q_s[lo_i:hi_i]
                for q in range(NCHUNK):
                    cnt[c, si, q, bi] = (qs == q).sum()
    cnt_sq = cnt.sum(axis=3)                                  # [NC, nsb, q]
    Trun = np.ceil(cnt_sq / P).astype(np.int64).max(axis=0)   # [nsb, q]
    Trun = np.maximum(Trun, 1)

    # slot layout + instance structure (global, core-agnostic)
    sb_meta = []
    tile_base = 0
    oh_base = 0
    for si, blist in enumerate(sblocks):
        segs = []               # per q: (tile_base_global, T)
        sb_tb = tile_base
        sb_ohb = oh_base
        pad_groups = []         # per tile_rel: list of (ohcol_rel, bi)
        agg = {bi: [] for bi in range(len(blist))}   # bi -> [(tile_rel, ohcol_rel)]
        inst_desc = []          # (tile_rel, bi) in oh column order
        for q in range(NCHUNK):
            T = int(Trun[si, q])
            segs.append((tile_base, T))
            # instance structure: union over cores of block spans
            # block bi span in run for core c: [off[c][bi], off[c][bi+1])
            offs = np.zeros((NC, len(blist) + 1), np.int64)
            for c in range(NC):
                offs[c, 1:] = np.cumsum(cnt[c, si, q, :len(blist)])
            for t in range(T):
                t_rel_global = tile_base - sb_tb + t
                s0, s1 = t * P, (t + 1) * P
                for bi in range(len(blist)):
                    hit = False
                    for c in range(NC):
                        if offs[c, bi] < s1 and offs[c, bi + 1] > s0:
                            hit = True
                            break
                    if hit:
                        inst_desc.append((t_rel_global, bi))
            tile_base += T
        S = tile_base - sb_tb
        Sx = len(inst_desc)
        oh_base += Sx
        pad_groups = [[] for _ in range(S)]
        for ohc, (t_rel, bi) in enumerate(inst_desc):
            pad_groups[t_rel].append((ohc, bi))
            agg[bi].append((t_rel, ohc))
        sb_meta.append(dict(base=sb_tb, S=S, ohbase=sb_ohb, Sx=Sx, segs=segs,
                            blocks=blist, b0=blist[0], inst=inst_desc,
                            pad_groups=pad_groups, agg=agg))
    Tsum = tile_base
    SxT = oh_base

    # per-core slot-value arrays
    ihC_w = np.zeros((NC, P, Tsum * 8), np.int16)
    dlx2d = np.zeros((NC, P, SxT), bf16)
    dlxT = np.zeros((NC, 1, SxT * P), bf16)
    oTh = np.zeros((NC, P, SxT * P), bf16)
    for c in range(NC):
        ihC = np.zeros(Tsum * P, np.int16)
        dlx = np.full(SxT * P, 255.0, np.float32)
        for si, blist in enumerate(sblocks):
            sb = sb_meta[si]
            for q in range(NCHUNK):
                tb, T = sb["segs"][q]
                # this core's edges for (sb, q), dst-sorted
                lo = c * NPC + blist[0] * P
                hi = c * NPC + blist[-1] * P + P
                lo_i, hi_i = np.searchsorted(dst_s, lo), np.searchsorted(dst_s, hi)
                m = q_s[lo_i:hi_i] == q
                es = src_s[lo_i:hi_i][m]
                ed = dst_s[lo_i:hi_i][m]
                n = len(es)
                assert n <= T * P, (n, T * P)
                s0 = tb * P
                # row idx within chunk q (pi_C layout, used by both phases)
                cs = es // NPC
                loc = es % NPC
                ihC[s0:s0 + n] = ((cs % 2) * NPC + (loc % P) * NB
                                  + loc // P).astype(np.int16)
                # dloc per instance column
                blk = (ed - c * NPC) // P - blist[0]          # bi of each edge
                dloc = ed - (c * NPC + (blist[0] + blk) * P)  # 0..127
                for ohc, (t_rel, bi) in enumerate(sb["inst"]):
                    pass
                # fill instance columns for this (sb, q)
                for t in range(T):
                    t_rel = tb - sb["base"] + t
                    e0, e1 = t * P, min((t + 1) * P, n)
                    if e0 >= n:
                        continue
                    for (ohc, bi) in sb["pad_groups"][t_rel]:
                        col0 = (sb["ohbase"] + ohc) * P
                        idx = np.arange(e0, e1)
                        sel = blk[idx] == bi
                        lanes = idx - t * P
                        vals = np.full(len(idx), 255.0, np.float32)
                        vals[sel] = dloc[idx[sel]]
                        dlx[col0 + lanes] = vals
        ihC_w[c] = _wrap16(ihC)
        dlx2d[c] = dlx.reshape(SxT, P).T.astype(bf16)
        dlxT[c, 0] = dlx.astype(bf16)
        oTh[c] = (np.arange(P, dtype=np.float32)[:, None]
                  == dlx[None, :]).astype(bf16)

    shared = {
        "W1aug": W1aug_b, "W2aug": W2aug_b, "b1rep": b1rep,
        "b2rep": b2rep, "iota": iota, "ident": ident,
        "iotac": np.arange(P, dtype=np.float32).reshape(P, 1),
        "onesk": np.ones((1, P), bf16),
    }
    in_maps = []
    for c in range(NC):
        m = dict(shared)
        m["xTloc"] = np.ascontiguousarray(xT[:, c * NPC:(c + 1) * NPC])
        m["ihsrcC"] = ihC_w[c]
        m["dlx2d"] = dlx2d[c]
        m["dlxT"] = dlxT[c]
        m["oTh"] = oTh[c]
        in_maps.append(m)

    meta = dict(cfg, R1=R1, RG=RG, HC=HC, NPC=NPC, NB=NB, NT=NT, Np=Np,
                CHB=CHB, TPC=TPC, RL2=RL2, Tsum=Tsum, SxT=SxT,
                sb_meta=sb_meta, SBG=SBG)
    return in_maps, meta


# ----------------------------------------------------------------------------
# device program
# ----------------------------------------------------------------------------

def _sub(ap, elem_off, dims):
    return bass.AP(ap.tensor, ap.offset + elem_off, [ap.ap[0], *list(dims)])


def build(meta, nc=None):
    N, F, H, C, CLS = meta["N"], meta["F"], meta["H"], meta["C"], meta["CLS"]
    NC, R1, RG, HC = meta["NC"], meta["R1"], meta["RG"], meta["HC"]
    NPC, NB, NT, Np = meta["NPC"], meta["NB"], meta["NT"], meta["Np"]
    CHB, TPC, RL2 = meta["CHB"], meta["TPC"], meta["RL2"]
    Tsum, SxT = meta["Tsum"], meta["SxT"]
    sb_meta = meta["sb_meta"]
    SBG = meta["SBG"]
    R2 = 4

    f32, bf16, i16 = mybir.dt.float32, mybir.dt.bfloat16, mybir.dt.int16

    if nc is None:
        nc = bacc.Bacc("TRN2", target_bir_lowering=False, debug=False,
                       num_devices=NC, num_swdge_queues=NQUEUE,
                       dynamic_dma_scratch_size=DMA_SCRATCH)

    qrr = [0]

    def gather_split(out_tile, rel, segT, elem, table, ix_tile):
        """Split a segment gather into <=MAXT-tile calls, round-robin queues."""
        done = 0
        while done < segT:
            tt = min(MAXT, segT - done)
            r = rel + done
            nc.gpsimd.dma_gather(
                bass.AP(out_tile[:].tensor, out_tile[:].offset + r * elem,
                        [out_tile[:].ap[0], [elem, tt], [1, elem]]),
                table,
                ix_tile[:, r * 8:(r + tt) * 8],
                tt * P, tt * P, elem,
                queue_num=qrr[0] % NQUEUE,
            )
            qrr[0] += 1
            done += tt

    xTl_d = nc.dram_tensor("xTloc", [F, NPC], bf16, kind="ExternalInput")
    W1aug_d = nc.dram_tensor("W1aug", [F, R1 + H], bf16, kind="ExternalInput")
    W2aug_d = nc.dram_tensor("W2aug", [HC, R2], bf16, kind="ExternalInput")
    b1rep_d = nc.dram_tensor("b1rep", [P, HC], bf16, kind="ExternalInput")
    b2rep_d = nc.dram_tensor("b2rep", [P, CLS], f32, kind="ExternalInput")
    iota_d = nc.dram_tensor("iota", [P, P], bf16, kind="ExternalInput")
    ident_d = nc.dram_tensor("ident", [P, P], bf16, kind="ExternalInput")
    ihC_d = nc.dram_tensor("ihsrcC", [P, Tsum * 8], i16, kind="ExternalInput")
    dlx2d_d = nc.dram_tensor("dlx2d", [P, SxT], bf16, kind="ExternalInput")
    dlxT_d = nc.dram_tensor("dlxT", [1, SxT * P], bf16, kind="ExternalInput")
    oTh_d = nc.dram_tensor("oTh", [P, SxT * P], bf16, kind="ExternalInput")
    iotac_d = nc.dram_tensor("iotac", [P, 1], f32, kind="ExternalInput")
    onesk_d = nc.dram_tensor("onesk", [1, P], bf16, kind="ExternalInput")
    out_d = nc.dram_tensor("out", [NPC, CLS], f32, kind="ExternalOutput")

    hloc = nc.dram_tensor("hloc", [NPC, RG], bf16, kind="Internal")
    htabS = nc.dram_tensor("htabS", [Np, RG], bf16, kind="Internal",
                           addr_space="Shared")
    h2loc64 = nc.dram_tensor("h2loc64", [NPC, RL2], f32, kind="Internal")
    h2tab64 = nc.dram_tensor("h2tab64", [Np, RL2], f32, kind="Internal",
                             addr_space="Shared")

    FA = min(P, F)
    FB = F - FA

    with tile.TileContext(nc) as tc:
        with tc.tile_pool(name="const", bufs=1) as cp:
            w1a = cp.tile([FA, R1 + H], bf16)
            nc.sync.dma_start(out=w1a[:], in_=W1aug_d[0:FA, :])
            w1b = cp.tile([FB, R1 + H], bf16)
            nc.sync.dma_start(out=w1b[:], in_=W1aug_d[FA:F, :])
            w2a = cp.tile([P, R2], bf16)
            nc.sync.dma_start(out=w2a[:], in_=W2aug_d[0:P, :])
            w2b = cp.tile([P, R2], bf16)
            nc.sync.dma_start(out=w2b[:], in_=W2aug_d[P:HC, :])
            b1s = cp.tile([P, HC], bf16)
            nc.sync.dma_start(out=b1s[:], in_=b1rep_d[:, :])
            b2s = cp.tile([P, CLS], f32)
            nc.sync.dma_start(out=b2s[:], in_=b2rep_d[:, :])
            iot = cp.tile([P, P], bf16)
            nc.sync.dma_start(out=iot[:], in_=iota_d[:, :])
            idn = cp.tile([P, P], bf16)
            nc.sync.dma_start(out=idn[:], in_=ident_d[:, :])
            dlc = cp.tile([P, SxT], bf16)
            nc.sync.dma_start(out=dlc[:], in_=dlx2d_d[:, :])
            iotc = cp.tile([P, 1], f32)
            nc.sync.dma_start(out=iotc[:], in_=iotac_d[:, :])
            onek = cp.tile([1, P], bf16)
            nc.sync.dma_start(out=onek[:], in_=onesk_d[:, :])
            # persistent SBUF state
            slocS = cp.tile([P, NB * H], bf16)       # a_dst of own nodes
            h2self = cp.tile([P, NB * R2], f32)      # own h2 rows
            vstage = cp.tile([P, NB * R2], f32)      # phase-C accumulators

            # ---------------- mini-pass: h table (own slice) -----------------
            with tc.tile_pool(name="pm", bufs=2) as pm, \
                 tc.tile_pool(name="psm", bufs=4, space="PSUM") as psm:
                for s in range(NPC // (SLAB * P)):          # 7 slabs
                    c0 = s * SLAB * P
                    xa = pm.tile([FA, SLAB * P], bf16, tag="xa")
                    nc.sync.dma_start(out=xa[:], in_=xTl_d[0:FA, c0:c0 + SLAB * P])
                    xb = pm.tile([FB, SLAB * P], bf16, tag="xb")
                    nc.sync.dma_start(out=xb[:], in_=xTl_d[FA:F, c0:c0 + SLAB * P])
                    stg = pm.tile([P, SLAB * RG], bf16, tag="stg")
                    for t in range(SLAB):
                        ph = psm.tile([P, R1 + H], f32, tag="ph")
                        nc.tensor.matmul(out=ph[:], lhsT=xa[:, t * P:(t + 1) * P],
                                         rhs=w1a[:], start=True, stop=False)
                        nc.tensor.matmul(out=ph[:], lhsT=xb[:, t * P:(t + 1) * P],
                                         rhs=w1b[:], start=False, stop=True)
                        nc.vector.tensor_copy(out=stg[:, t * RG:t * RG + R1],
                                              in_=ph[:, :R1])
                        nc.vector.tensor_copy(
                            out=slocS[:, (s * SLAB + t) * H:(s * SLAB + t + 1) * H],
                            in_=ph[:, R1:R1 + H])
                    nc.sync.dma_start(
                        out=bass.AP(hloc, s * SLAB * RG,
                                    [[NB * RG, P], [RG, SLAB], [1, RG]]),
                        in_=stg[:])

            # ---------------- AllGather h table ------------------------------
            nc.gpsimd.collective_compute(
                "AllGather", mybir.AluOpType.bypass,
                replica_groups=[list(range(NC))],
                ins=[hloc[:, :]], outs=[htabS[:, :]])

            # ---------------- Phase B: L1 edge pass --------------------------
            with tc.tile_pool(name="pbg", bufs=3) as pbg, \
                 tc.tile_pool(name="pbo", bufs=2) as pbo, \
                 tc.tile_pool(name="pbb", bufs=3) as pbb, \
                 tc.tile_pool(name="psb", bufs=4, space="PSUM") as psb, \
                 tc.tile_pool(name="psp", bufs=2, space="PSUM") as psp, \
                 tc.tile_pool(name="pst", bufs=1, space="PSUM") as pst, \
                 tc.tile_pool(name="psh", bufs=1, space="PSUM") as psh:
                for sb in sb_meta:
                    base, S, Sx, ohb = sb["base"], sb["S"], sb["Sx"], sb["ohbase"]
                    blist, b0 = sb["blocks"], sb["b0"]
                    nblk = len(blist)
                    g = pbg.tile([P, S * RG], bf16, tag="g")
                    ixs = pbg.tile([P, S * 8], i16, tag="ixs")
                    nc.sync.dma_start(out=ixs[:],
                                      in_=ihC_d[:, base * 8:(base + S) * 8])
                    for q in range(NCHUNK):
                        tb, T = sb["segs"][q]
                        gather_split(g, tb - base, T, RG,
                                     htabS[q * CHB:(q + 1) * CHB, :], ixs)
                    selfh = pbg.tile([P, SBG * RG], bf16, tag="selfh")
                    nc.sync.dma_start(
                        out=selfh[:, :nblk * RG],
                        in_=bass.AP(hloc, b0 * RG,
                                    [[NB * RG, P], [1, nblk * RG]]))
                    # oT: [dst_local, inst_col] one-hot (host-precomputed);
                    # split load so early pad matmuls start at half-load
                    oT = pbo.tile([P, Sx * P], bf16, tag="oT")
                    h1 = (Sx // 2) * P
                    nc.sync.dma_start(out=oT[:, :h1],
                                      in_=oTh_d[:, ohb * P:ohb * P + h1])
                    nc.sync.dma_start(out=oT[:, h1:Sx * P],
                                      in_=oTh_d[:, ohb * P + h1:(ohb + Sx) * P])
                    # per-slot a_dst via oT matmuls -> PSUM [P, S*H]
                    pad = psp.tile([P, S * H], f32, tag="pad")
                    for t in range(S):
                        grp = sb["pad_groups"][t]
                        for gi, (ohc, bi) in enumerate(grp):
                            nc.tensor.matmul(
                                out=pad[:, t * H:(t + 1) * H],
                                lhsT=oT[:, ohc * P:(ohc + 1) * P],
                                rhs=slocS[:, (b0 + bi) * H:(b0 + bi + 1) * H],
                                start=(gi == 0), stop=(gi == len(grp) - 1),
                                skip_group_check=True)
                    # ex = exp(lrelu(asrc+adst))  [P, S*H] f32
                    ex = pbb.tile([P, S * H], f32, tag="ex")
                    nc.vector.tensor_tensor(
                        out=ex[:].rearrange("p (t h) -> p t h", t=S),
                        in0=_sub(g[:], HC, [[RG, S], [1, H]]),
                        in1=pad[:].rearrange("p (t h) -> p t h", t=S),
                        op=mybir.AluOpType.add)
                    tmp = pbb.tile([P, S * H], f32, tag="tmp")
                    nc.vector.tensor_scalar_mul(out=tmp[:], in0=ex[:], scalar1=NEG)
                    nc.vector.tensor_tensor(out=ex[:], in0=ex[:], in1=tmp[:],
                                            op=mybir.AluOpType.max)
                    nc.scalar.activation(out=ex[:], in_=ex[:],
                                         func=mybir.ActivationFunctionType.Exp)
                    exb = pbb.tile([P, S * H], bf16, tag="exb")
                    nc.vector.tensor_copy(out=exb[:], in_=ex[:])
                    # msg in-place: cols 0:HC *= ex ; cols HC:HC+H = ex
                    nc.vector.tensor_tensor(
                        out=_sub(g[:], 0, [[RG, S], [C, H], [1, C]]),
                        in0=_sub(g[:], 0, [[RG, S], [C, H], [1, C]]),
                        in1=_sub(exb[:], 0, [[H, S], [1, H], [0, C]]),
                        op=mybir.AluOpType.mult)
                    nc.vector.tensor_copy(
                        out=_sub(g[:], HC, [[RG, S], [1, H]]),
                        in_=exb[:].rearrange("p (t h) -> p t h", t=S))
                    # oh: [slot, inst_col] one-hot
                    oh = pbo.tile([P, Sx * P], bf16, tag="oh")
                    nc.vector.tensor_tensor(
                        out=oh[:].rearrange("p (t q) -> p t q", t=Sx),
                        in0=_sub(dlc[:], ohb, [[1, Sx], [0, P]]),
                        in1=_sub(iot[:], 0, [[0, Sx], [1, P]]),
                        op=mybir.AluOpType.is_equal)
                    # self-loop messages (batched over blocks)
                    lS = pbb.tile([P, SBG * H], f32, tag="lS")
                    nc.vector.tensor_tensor(
                        out=lS[:, :nblk * H].rearrange("p (b h) -> p b h", b=nblk),
                        in0=_sub(selfh[:], HC, [[RG, nblk], [1, H]]),
                        in1=_sub(slocS[:], b0 * H, [[H, nblk], [1, H]]),
                        op=mybir.AluOpType.add)
                    tS = pbb.tile([P, SBG * H], f32, tag="tS")
                    nc.vector.tensor_scalar_mul(out=tS[:], in0=lS[:], scalar1=NEG)
                    nc.vector.tensor_tensor(out=lS[:], in0=lS[:], in1=tS[:],
                                            op=mybir.AluOpType.max)
                    nc.scalar.activation(out=lS[:], in_=lS[:],
                                         func=mybir.ActivationFunctionType.Exp)
                    selfm = pbb.tile([P, SBG * (R1 + 4)], bf16, tag="selfm")
                    RS = R1 + 4
                    nc.vector.tensor_tensor(
                        out=_sub(selfm[:], 0, [[RS, nblk], [C, H], [1, C]]),
                        in0=_sub(selfh[:], 0, [[RG, nblk], [C, H], [1, C]]),
                        in1=_sub(lS[:], 0, [[H, nblk], [1, H], [0, C]]),
                        op=mybir.AluOpType.mult)
                    nc.vector.tensor_copy(
                        out=_sub(selfm[:], HC, [[RS, nblk], [1, H]]),
                        in_=lS[:, :nblk * H].rearrange("p (b h) -> p b h", b=nblk))
                    # per-block aggregation + batched epilogue
                    psoS = pbb.tile([P, SBG * RS], f32, tag="psoS")
                    for bi in range(nblk):
                        runs = sb["agg"][bi]
                        pso = psb.tile([P, R1], f32, tag="pso")
                        for ri, (t_rel, ohc) in enumerate(runs):
                            nc.tensor.matmul(
                                out=pso[:],
                                lhsT=oh[:, ohc * P:(ohc + 1) * P],
                                rhs=g[:, t_rel * RG:t_rel * RG + R1],
                                start=(ri == 0), stop=(ri == len(runs) - 1))
                        nc.vector.tensor_tensor(
                            out=psoS[:, bi * RS:bi * RS + R1],
                            in0=pso[:],
                            in1=selfm[:, bi * RS:bi * RS + R1],
                            op=mybir.AluOpType.add)
                    den = pbb.tile([P, SBG * H], f32, tag="den")
                    nc.vector.tensor_scalar_max(
                        out=den[:, :nblk * H].rearrange("p (b h) -> p b h", b=nblk),
                        in0=_sub(psoS[:], HC, [[RS, nblk], [1, H]]),
                        scalar1=1e-20)
                    rde = pbb.tile([P, SBG * H], f32, tag="rde")
                    nc.vector.reciprocal(out=rde[:], in_=den[:])
                    o1 = pbb.tile([P, SBG * HC], bf16, tag="o1")
                    nc.vector.tensor_tensor(
                        out=o1[:].rearrange("p (b h c) -> p b h c", b=SBG, h=H),
                        in0=_sub(psoS[:], 0, [[RS, SBG], [C, H], [1, C]]),
                        in1=_sub(rde[:], 0, [[H, SBG], [1, H], [0, C]]),
                        op=mybir.AluOpType.mult)
                    nc.vector.tensor_tensor(
                        out=o1[:].rearrange("p (b k) -> p b k", b=SBG),
                        in0=o1[:].rearrange("p (b k) -> p b k", b=SBG),
                        in1=_sub(b1s[:], 0, [[0, SBG], [1, HC]]),
                        op=mybir.AluOpType.add)
                    nc.scalar.activation(out=o1[:], in_=o1[:],
                                         func=mybir.ActivationFunctionType.Relu)
                    # h2 = relu(o1) @ W2aug via PE transposes
                    ptr = pst.tile([P, 2 * SBG * P], bf16, tag="ptr")
                    for bi in range(nblk):
                        for k in range(2):
                            nc.tensor.transpose(
                                out=ptr[:, (bi * 2 + k) * P:(bi * 2 + k + 1) * P],
                                in_=o1[:, bi * HC + k * P:bi * HC + (k + 1) * P],
                                identity=idn[:])
                    rT = pbb.tile([P, 2 * SBG * P], bf16, tag="rT")
                    nc.vector.tensor_copy(out=rT[:, :nblk * 2 * P],
                                          in_=ptr[:, :nblk * 2 * P])
                    ph2 = psh.tile([P, SBG * R2], f32, tag="ph2")
                    for bi in range(nblk):
                        nc.tensor.matmul(out=ph2[:, bi * R2:(bi + 1) * R2],
                                         lhsT=rT[:, bi * 2 * P:(bi * 2 + 1) * P],
                                         rhs=w2a[:], start=True, stop=False,
                                         skip_group_check=True)
                        nc.tensor.matmul(out=ph2[:, bi * R2:(bi + 1) * R2],
                                         lhsT=rT[:, (bi * 2 + 1) * P:(bi * 2 + 2) * P],
                                         rhs=w2b[:], start=False, stop=True,
                                         skip_group_check=True)
                    nc.vector.tensor_copy(out=h2self[:, b0 * R2:(b0 + nblk) * R2],
                                          in_=ph2[:, :nblk * R2])
                    h2st = pbb.tile([P, SBG * RL2], f32, tag="h2st")
                    nc.vector.tensor_copy(
                        out=_sub(h2st[:], 0, [[RL2, nblk], [1, R2]]),
                        in_=ph2[:, :nblk * R2].rearrange("p (b r) -> p b r", b=nblk))
                    nc.sync.dma_start(
                        out=bass.AP(h2loc64, b0 * RL2,
                                    [[NB * RL2, P], [1, nblk * RL2]]),
                        in_=h2st[:, :nblk * RL2])

            # ---------------- AllGather ---------------------------------------
            nc.gpsimd.collective_compute(
                "AllGather", mybir.AluOpType.bypass,
                replica_groups=[list(range(NC))],
                ins=[h2loc64[:, :]], outs=[h2tab64[:, :]])

            # ---------------- Phase C: L2 edge pass --------------------------
            with tc.tile_pool(name="pcg", bufs=4) as pcg, \
                 tc.tile_pool(name="pco", bufs=3) as pco, \
                 tc.tile_pool(name="pcb", bufs=2) as pcb, \
                 tc.tile_pool(name="psc", bufs=4, space="PSUM") as psc, \
                 tc.tile_pool(name="psp2", bufs=2, space="PSUM") as psp2, \
                 tc.tile_pool(name="psk2", bufs=2, space="PSUM") as psk2:
                for sb in sb_meta:
                    base, S, Sx, ohb = sb["base"], sb["S"], sb["Sx"], sb["ohbase"]
                    blist, b0 = sb["blocks"], sb["b0"]
                    nblk = len(blist)
                    g2 = pcg.tile([P, S * RL2], f32, tag="g2")
                    ixs = pcg.tile([P, S * 8], i16, tag="ixs2")
                    nc.sync.dma_start(out=ixs[:],
                                      in_=ihC_d[:, base * 8:(base + S) * 8])
                    for q in range(NCHUNK):
                        tb, T = sb["segs"][q]
                        gather_split(g2, tb - base, T, RL2,
                                     h2tab64[q * CHB:(q + 1) * CHB, :], ixs)
                    dlT = pcg.tile([1, Sx * P], bf16, tag="dlT2")
                    nc.sync.dma_start(out=dlT[:],
                                      in_=dlxT_d[0:1, ohb * P:(ohb + Sx) * P])
                    oT = pco.tile([P, Sx * P], bf16, tag="oT2")
                    for st in range(0, Sx * P, 512):
                        w = min(512, Sx * P - st)
                        stp = psk2.tile([P, 512], f32, tag="stp2")
                        nc.tensor.matmul(out=stp[:, :w], lhsT=onek[:],
                                         rhs=dlT[0:1, st:st + w],
                                         start=True, stop=True)
                        nc.vector.tensor_tensor(
                            out=oT[:, st:st + w],
                            in0=iotc[:, 0:1].to_broadcast([P, w]),
                            in1=stp[:, :w],
                            op=mybir.AluOpType.is_equal)
                    adw2 = pcb.tile([P, SBG], bf16, tag="adw2")
                    nc.vector.tensor_copy(
                        out=adw2[:, :nblk],
                        in_=_sub(h2self[:], b0 * R2 + 3, [[R2, nblk]]))
                    pad2 = psp2.tile([P, S], f32, tag="pad2")
                    for t in range(S):
                        grp = sb["pad_groups"][t]
                        for gi, (ohc, bi) in enumerate(grp):
                            nc.tensor.matmul(
                                out=pad2[:, t:t + 1],
                                lhsT=oT[:, ohc * P:(ohc + 1) * P],
                                rhs=adw2[:, bi:bi + 1],
                                start=(gi == 0), stop=(gi == len(grp) - 1),
                                skip_group_check=True)
                    ex2 = pcb.tile([P, S], f32, tag="ex2")
                    nc.vector.tensor_tensor(
                        out=ex2[:],
                        in0=_sub(g2[:], CLS, [[RL2, S]]),
                        in1=pad2[:],
                        op=mybir.AluOpType.add)
                    tm2 = pcb.tile([P, S], f32, tag="tm2")
                    nc.vector.tensor_scalar_mul(out=tm2[:], in0=ex2[:], scalar1=NEG)
                    nc.vector.tensor_tensor(out=ex2[:], in0=ex2[:], in1=tm2[:],
                                            op=mybir.AluOpType.max)
                    nc.scalar.activation(out=ex2[:], in_=ex2[:],
                                         func=mybir.ActivationFunctionType.Exp)
                    m2 = pcb.tile([P, S * R2], bf16, tag="m2")
                    nc.vector.tensor_tensor(
                        out=_sub(m2[:], 0, [[R2, S], [1, CLS]]),
                        in0=_sub(g2[:], 0, [[RL2, S], [1, CLS]]),
                        in1=_sub(ex2[:], 0, [[1, S], [0, CLS]]),
                        op=mybir.AluOpType.mult)
                    nc.vector.tensor_copy(
                        out=_sub(m2[:], CLS, [[R2, S], [1, 2]]),
                        in_=_sub(ex2[:], 0, [[1, S], [0, 2]]))
                    oh = pco.tile([P, Sx * P], bf16, tag="oh2")
                    nc.vector.tensor_tensor(
                        out=oh[:].rearrange("p (t q) -> p t q", t=Sx),
                        in0=_sub(dlc[:], ohb, [[1, Sx], [0, P]]),
                        in1=_sub(iot[:], 0, [[0, Sx], [1, P]]),
                        op=mybir.AluOpType.is_equal)
                    # self-loop L2 messages
                    l2S = pcb.tile([P, SBG], f32, tag="l2S")
                    nc.vector.tensor_tensor(
                        out=l2S[:, :nblk],
                        in0=_sub(h2self[:], b0 * R2 + 2, [[R2, nblk]]),
                        in1=_sub(h2self[:], b0 * R2 + 3, [[R2, nblk]]),
                        op=mybir.AluOpType.add)
                    t2S = pcb.tile([P, SBG], f32, tag="t2S")
                    nc.vector.tensor_scalar_mul(out=t2S[:], in0=l2S[:], scalar1=NEG)
                    nc.vector.tensor_tensor(out=l2S[:], in0=l2S[:], in1=t2S[:],
                                            op=mybir.AluOpType.max)
                    nc.scalar.activation(out=l2S[:], in_=l2S[:],
                                         func=mybir.ActivationFunctionType.Exp)
                    sm2 = pcb.tile([P, SBG * R2], f32, tag="sm2")
                    nc.vector.tensor_tensor(
                        out=_sub(sm2[:], 0, [[R2, nblk], [1, CLS]]),
                        in0=_sub(h2self[:], b0 * R2, [[R2, nblk], [1, CLS]]),
                        in1=_sub(l2S[:], 0, [[1, nblk], [0, CLS]]),
                        op=mybir.AluOpType.mult)
                    nc.vector.tensor_copy(
                        out=_sub(sm2[:], CLS, [[R2, nblk], [1, 2]]),
                        in_=_sub(l2S[:], 0, [[1, nblk], [0, 2]]))
                    ps2 = psc.tile([P, SBG * R2], f32, tag="ps2")
                    for bi in range(nblk):
                        runs = sb["agg"][bi]
                        for ri, (t_rel, ohc) in enumerate(runs):
                            nc.tensor.matmul(
                                out=ps2[:, bi * R2:(bi + 1) * R2],
                                lhsT=oh[:, ohc * P:(ohc + 1) * P],
                                rhs=m2[:, t_rel * R2:(t_rel + 1) * R2],
                                start=(ri == 0), stop=(ri == len(runs) - 1),
                                skip_group_check=True)
                    nc.vector.tensor_tensor(
                        out=vstage[:, b0 * R2:(b0 + nblk) * R2],
                        in0=ps2[:, :nblk * R2],
                        in1=sm2[:, :nblk * R2],
                        op=mybir.AluOpType.add)

                # ------------- final: normalize + log-softmax ----------------
                den2 = pcb.tile([P, NB], f32, tag="den2")
                nc.vector.tensor_scalar_max(out=den2[:],
                                            in0=_sub(vstage[:], CLS, [[R2, NB]]),
                                            scalar1=1e-20)
                rd2 = pcb.tile([P, NB], f32, tag="rd2")
                nc.vector.reciprocal(out=rd2[:], in_=den2[:])
                v = pcb.tile([P, NB * CLS], f32, tag="v")
                nc.vector.tensor_tensor(
                    out=v[:].rearrange("p (b k) -> p b k", b=NB),
                    in0=_sub(vstage[:], 0, [[R2, NB], [1, CLS]]),
                    in1=_sub(rd2[:], 0, [[1, NB], [0, CLS]]),
                    op=mybir.AluOpType.mult)
                nc.vector.tensor_tensor(
                    out=v[:].rearrange("p (b k) -> p b k", b=NB),
                    in0=v[:].rearrange("p (b k) -> p b k", b=NB),
                    in1=_sub(b2s[:], 0, [[0, NB], [1, CLS]]),
                    op=mybir.AluOpType.add)
                mx = pcb.tile([P, NB], f32, tag="mx")
                nc.vector.tensor_tensor(out=mx[:],
                                        in0=_sub(v[:], 0, [[CLS, NB]]),
                                        in1=_sub(v[:], 1, [[CLS, NB]]),
                                        op=mybir.AluOpType.max)
                u = pcb.tile([P, NB * CLS], f32, tag="u")
                nc.vector.tensor_tensor(
                    out=u[:].rearrange("p (b k) -> p b k", b=NB),
                    in0=v[:].rearrange("p (b k) -> p b k", b=NB),
                    in1=_sub(mx[:], 0, [[1, NB], [0, CLS]]),
                    op=mybir.AluOpType.subtract)
                nc.scalar.activation(out=u[:], in_=u[:],
                                     func=mybir.ActivationFunctionType.Exp)
                sm = pcb.tile([P, NB], f32, tag="sm")
                nc.vector.tensor_tensor(out=sm[:],
                                        in0=_sub(u[:], 0, [[CLS, NB]]),
                                        in1=_sub(u[:], 1, [[CLS, NB]]),
                                        op=mybir.AluOpType.add)
                ls = pcb.tile([P, NB], f32, tag="ls")
                nc.scalar.activation(out=ls[:], in_=sm[:],
                                     func=mybir.ActivationFunctionType.Ln)
                nc.vector.tensor_tensor(out=ls[:], in0=ls[:], in1=mx[:],
                                        op=mybir.AluOpType.add)
                res = pcb.tile([P, NB * CLS], f32, tag="res")
                nc.vector.tensor_tensor(
                    out=res[:].rearrange("p (b k) -> p b k", b=NB),
                    in0=v[:].rearrange("p (b k) -> p b k", b=NB),
                    in1=_sub(ls[:], 0, [[1, NB], [0, CLS]]),
                    op=mybir.AluOpType.subtract)
                nc.sync.dma_start(
                    out=bass.AP(out_d, 0, [[NB * CLS, P], [1, NB * CLS]]),
                    in_=res[:])
    nc.finalize()
    return nc


def install_ntff_hook(so_path="/opt/axon/libaxon_pjrt.so"):
    import types
    import ctypes
    import contextlib
    import antenv

    if getattr(antenv, "axon_hooks", None) is not None:
        return
    lib = ctypes.CDLL(so_path)
    if not hasattr(lib, "axon_start_nrt_profile"):
        return
    lib.axon_start_nrt_profile.argtypes = [ctypes.POINTER(ctypes.c_int64),
                                           ctypes.c_size_t]
    lib.axon_start_nrt_profile.restype = ctypes.c_int64
    lib.axon_stop_nrt_profile.argtypes = [ctypes.c_char_p]
    lib.axon_stop_nrt_profile.restype = ctypes.c_int64

    @contextlib.contextmanager
    def _hook(output_dir, device_ids):
        import jax
        jax.devices()
        if device_ids:
            ids = (ctypes.c_int64 * len(device_ids))(*device_ids)
            rc = lib.axon_start_nrt_profile(ids, len(device_ids))
        else:
            rc = lib.axon_start_nrt_profile(None, 0)
        if rc != 0:
            raise RuntimeError(f"axon_start_nrt_profile rc={rc}")
        try:
            yield
        finally:
            n = lib.axon_stop_nrt_profile(str(output_dir).encode())
            print(f"ntff profile: {n} file(s) written to {output_dir}")

    mod = types.ModuleType("antenv.axon_hooks")
    _reg = [_hook]
    mod.set_axon_ntff_profile_hook = lambda h: _reg.__setitem__(0, h)
    mod.get_axon_ntff_profile_hook = lambda: _reg[0]
    sys.modules["antenv.axon_hooks"] = mod
    antenv.axon_hooks = mod


def run(inputs, cfg, trace=False, **kwargs):
    if trace:
        install_ntff_hook()
    in_maps, meta = prep(inputs, cfg)
    nc = build(meta)
    res = bass_utils.run_bass_kernel_spmd(
        nc, in_maps, core_ids=list(range(cfg["NC"])), trace=trace, **kwargs)
    NPC, NB, N = meta["NPC"], meta["NB"], meta["N"]
    parts = []
    for c in range(cfg["NC"]):
        r = np.asarray(res.results[c]["out"])          # [NPC, 2], (p, b) order
        r = r.reshape(P, NB, cfg["CLS"]).transpose(1, 0, 2).reshape(NPC, cfg["CLS"])
        parts.append(r)
    out = np.concatenate(parts, axis=0)[:N]
    return out, res


# ----------------------------------------------------------------------------
# harness entry point
# ----------------------------------------------------------------------------

_CFG = dict(N=100000, F=165, H=4, C=64, CLS=2, NC=8, SBG=4)


def kernel(**inputs):
    """Full (unsharded) inputs -> full [N, 2] float32 log-softmax output.

    Shards edges by destination-node range across the 8 NeuronCores,
    compiles and runs the Bass/Tile kernel via run_bass_kernel_spmd,
    and reassembles the per-core output slices.
    """
    out, _ = run(inputs, _CFG, trace=False)
    return np.ascontiguousarray(out.astype(np.float32))


# revision 65
# speedup vs baseline: 1.1286x; 1.0075x over previous
"""GAT 2-layer message-passing network on 8 TRN2 NeuronCores (Bass/Tile).

v3: restructured around the v2 trace findings (phase A Sync-issue-bound,
phases B/C gather-DGE + small-op bound, 337us repack of tiny descriptors).

Strategy (dst-sharded, uniform NPC=12544 with tail pad nodes):
 - Core c owns nodes [c*12544, (c+1)*12544) (core 7 has 352 pad nodes) and
   all real (non-self-loop) edges into them. Self loops are handled
   analytically on-chip (diagonal add), NOT via gather slots -- this cuts
   slot padding sharply.
 - Each core computes h only for its OWN nodes (mini-pass, 7 slab loads /
   stores with 128 large descriptors each, local pi rows p*98+b), then one
   AllGather replicates hloc into the Shared table htabS [100352, 768B] at
   rows pi(n) = c*12544 + (nl%128)*98 + nl//128. The layer-2 table h2tab64
   [100352, 256B] uses the same pi, so BOTH edge phases share one slot
   geometry, one gather-chunk function q = src//25088 (int16-safe indices),
   and one index array; only the table/row size differ.
 - Slots: per superblock (4 dst blocks) x chunk runs, tiles of 128 slots may
   span blocks; boundary tiles get one one-hot column-set per touched block
   (dloc sentinel 255 masks foreign slots), so padding is per-(sb,q) only.
 - Per sb: gather 768B rows; a_dst per slot via oT one-hot matmuls from
   SBUF-resident slocS; ex=exp(lrelu(asrc+adst)); msg in-place; per-block
   PSUM aggregation via oh one-hot matmuls; self-loop contribution added as
   vector ops from an hloc row load; batched (per-sb) normalize + bias +
   relu + W2 matmul; h2 rows staged and stored in pi_C layout.
 - AllGather h2loc64 [12544,64]f32 -> Shared h2tab64 [100352,64].
 - Phase C: same slots, 256B-row gathers, batched epilogue into vstage;
   single final log-softmax over all blocks and one pi-ordered output store.
"""
import sys

if "/opt/trn_rl_repo" not in sys.path:
    sys.path.insert(0, "/opt/trn_rl_repo")

import math
import numpy as np
import ml_dtypes

import concourse.bass as bass
import concourse.bacc as bacc
import concourse.mybir as mybir
import concourse.tile as tile
from concourse import bass_utils

P = 128
NEG = 0.2
NCHUNK = 4
NQUEUE = 4
SLAB = 14                 # phase-A tiles per slab (14 | 196)
MAXT = 7                  # ring cap; gather_split balances call sizes evenly
DMA_SCRATCH = 16384       # SWDGE carveout bytes/partition (1024 descs/queue)

# Tile's DMASW sem-lane assignment round-robins over all Pool DMAs, which
# breaks the per-lane FIFO assumption when SWDGE DMAs run on multiple queues
# (out-of-order completion across queues under one counting sem). Patch the
# lane choice to lane == queue_num: per-lane FIFO again holds (each HW ring
# drains in order), and queues get independent lanes.
from concourse import tile_sem_assignment as _tsa  # noqa: E402

if not getattr(_tsa.TileClockTick, "_qaware_patched", False):
    _orig_assign_tick = _tsa.TileClockTick._assign_tick

    def _qaware_assign_tick(self, inst):
        if (isinstance(inst, _tsa.DMAInst)
                and inst.engine == mybir.EngineType.Pool):
            self.next_sw_dma_idx = getattr(inst, "queue_num", 0) or 0
        return _orig_assign_tick(self, inst)

    _tsa.TileClockTick._assign_tick = _qaware_assign_tick
    _tsa.TileClockTick._qaware_patched = True


def _wrap16(flat):
    """[n] -> [128, n//16] wrapped in 16 partitions, replicated x8."""
    w = flat.reshape(-1, 16).T
    return np.tile(w, (8, 1))


# ----------------------------------------------------------------------------
# host-side data prep
# ----------------------------------------------------------------------------

def prep(inputs, cfg):
    N, F, H, C, CLS, NC = cfg["N"], cfg["F"], cfg["H"], cfg["C"], cfg["CLS"], cfg["NC"]
    SBG = cfg.get("SBG", 4)
    x = np.asarray(inputs["x"], np.float32)
    ei = np.asarray(inputs["edge_index"])
    W1 = np.asarray(inputs["W1"], np.float32)
    as1 = np.asarray(inputs["att_src1"], np.float32)
    ad1 = np.asarray(inputs["att_dst1"], np.float32)
    b1 = np.asarray(inputs["b1"], np.float32)
    W2 = np.asarray(inputs["W2"], np.float32)
    as2 = np.asarray(inputs["att_src2"], np.float32)
    ad2 = np.asarray(inputs["att_dst2"], np.float32)
    b2 = np.asarray(inputs["b2"], np.float32)

    HC = H * C                        # 256
    R1 = HC + H                       # gathered live row: [h | asrc]
    RG = 128 * math.ceil((R1 + H) / 128)  # 384 bf16 elems (768B rows)
    NPC, NB = 12544, 98
    NT = 784
    Np = NT * P                       # 100352
    CHB = Np // NCHUNK                # 25088 = 196*128 = 2*NPC
    TPC = CHB // P                    # 196 tiles per chunk
    RL2 = 64                          # f32 row elems for L2 table (256B)

    # ---- weights / constants -------------------------------------------------
    W1r = W1.reshape(F, H, C)
    Wsrc = np.einsum("fhc,hc->fh", W1r, as1)
    Wdst = np.einsum("fhc,hc->fh", W1r, ad1)
    W1aug = np.concatenate([W1, Wsrc, Wdst], axis=1)          # [F, 264]
    Wsrc2 = W2 @ as2.reshape(CLS, 1)
    Wdst2 = W2 @ ad2.reshape(CLS, 1)
    W2aug = np.concatenate([W2, Wsrc2, Wdst2], axis=1)        # [HC, 4]

    bf16 = ml_dtypes.bfloat16
    xT = np.zeros((F, Np), dtype=bf16)
    xT[:, :N] = x.T.astype(bf16)
    W1aug_b = W1aug.astype(bf16)
    W2aug_b = W2aug.astype(bf16)
    b1rep = np.tile(b1[None, :], (P, 1)).astype(bf16)
    b2rep = np.tile(b2[None, :], (P, 1)).astype(np.float32)
    iota = np.tile(np.arange(P, dtype=np.float32)[None, :], (P, 1)).astype(bf16)
    ident = np.eye(P, dtype=bf16)

    # ---- edges (real only; self loops handled on-chip) -----------------------
    src_all = np.asarray(ei[0], np.int64)
    dst_all = np.asarray(ei[1], np.int64)
    order = np.argsort(dst_all, kind="stable")
    src_s = src_all[order]
    dst_s = dst_all[order]
    q_s = src_s // CHB                                        # phase chunk

    # superblocks of dst blocks
    sblocks = [list(range(i, min(i + SBG, NB))) for i in range(0, NB, SBG)]

    # per-core, per-(sb, q, block) counts
    nsb = len(sblocks)
    cnt = np.zeros((NC, nsb, NCHUNK, SBG), np.int64)
    for c in range(NC):
        for si, blist in enumerate(sblocks):
            for bi, b in enumerate(blist):
                lo = c * NPC + b * P
                lo_i, hi_i = np.searchsorted(dst_s, lo), np.searchsorted(dst_s, lo + P)
                qs = q_s[lo_i:hi_i]
                for q in range(NCHUNK):
                    cnt[c, si, q, bi] = (qs == q).sum()
    cnt_sq = cnt.sum(axis=3)                                  # [NC, nsb, q]
    Trun = np.ceil(cnt_sq / P).astype(np.int64).max(axis=0)   # [nsb, q]
    Trun = np.maximum(Trun, 1)

    # slot layout + instance structure (global, core-agnostic)
    sb_meta = []
    tile_base = 0
    oh_base = 0
    for si, blist in enumerate(sblocks):
        segs = []               # per q: (tile_base_global, T)
        sb_tb = tile_base
        sb_ohb = oh_base
        pad_groups = []         # per tile_rel: list of (ohcol_rel, bi)
        agg = {bi: [] for bi in range(len(blist))}   # bi -> [(tile_rel, ohcol_rel)]
        inst_desc = []          # (tile_rel, bi) in oh column order
        for q in range(NCHUNK):
            T = int(Trun[si, q])
            segs.append((tile_base, T))
            # instance structure: union over cores of block spans
            # block bi span in run for core c: [off[c][bi], off[c][bi+1])
            offs = np.zeros((NC, len(blist) + 1), np.int64)
            for c in range(NC):
                offs[c, 1:] = np.cumsum(cnt[c, si, q, :len(blist)])
            for t in range(T):
                t_rel_global = tile_base - sb_tb + t
                s0, s1 = t * P, (t + 1) * P
                for bi in range(len(blist)):
                    hit = False
                    for c in range(NC):
                        if offs[c, bi] < s1 and offs[c, bi + 1] > s0:
                            hit = True
                            break
                    if hit:
                        inst_desc.append((t_rel_global, bi))
            tile_base += T
        S = tile_base - sb_tb
        Sx = len(inst_desc)
        oh_base += Sx
        pad_groups = [[] for _ in range(S)]
        for ohc, (t_rel, bi) in enumerate(inst_desc):
            pad_groups[t_rel].append((ohc, bi))
            agg[bi].append((t_rel, ohc))
        sb_meta.append(dict(base=sb_tb, S=S, ohbase=sb_ohb, Sx=Sx, segs=segs,
                            blocks=blist, b0=blist[0], inst=inst_desc,
                            pad_groups=pad_groups, agg=agg))
    Tsum = tile_base
    SxT = oh_base

    # per-core slot-value arrays
    ihC_w = np.zeros((NC, P, Tsum * 8), np.int16)
    dlx2d = np.zeros((NC, P, SxT), bf16)
    dlxT = np.zeros((NC, 1, SxT * P), bf16)
    oTh = np.zeros((NC, P, SxT * P), bf16)
    for c in range(NC):
        ihC = np.zeros(Tsum * P, np.int16)
        dlx = np.full(SxT * P, 255.0, np.float32)
        for si, blist in enumerate(sblocks):
            sb = sb_meta[si]
            for q in range(NCHUNK):
                tb, T = sb["segs"][q]
                # this core's edges for (sb, q), dst-sorted
                lo = c * NPC + blist[0] * P
                hi = c * NPC + blist[-1] * P + P
                lo_i, hi_i = np.searchsorted(dst_s, lo), np.searchsorted(dst_s, hi)
                m = q_s[lo_i:hi_i] == q
                es = src_s[lo_i:hi_i][m]
                ed = dst_s[lo_i:hi_i][m]
                n = len(es)
                assert n <= T * P, (n, T * P)
                s0 = tb * P
                # row idx within chunk q (pi_C layout, used by both phases)
                cs = es // NPC
                loc = es % NPC
                ihC[s0:s0 + n] = ((cs % 2) * NPC + (loc % P) * NB
                                  + loc // P).astype(np.int16)
                # dloc per instance column
                blk = (ed - c * NPC) // P - blist[0]          # bi of each edge
                dloc = ed - (c * NPC + (blist[0] + blk) * P)  # 0..127
                for ohc, (t_rel, bi) in enumerate(sb["inst"]):
                    pass
                # fill instance columns for this (sb, q)
                for t in range(T):
                    t_rel = tb - sb["base"] + t
                    e0, e1 = t * P, min((t + 1) * P, n)
                    if e0 >= n:
                        continue
                    for (ohc, bi) in sb["pad_groups"][t_rel]:
                        col0 = (sb["ohbase"] + ohc) * P
                        idx = np.arange(e0, e1)
                        sel = blk[idx] == bi
                        lanes = idx - t * P
                        vals = np.full(len(idx), 255.0, np.float32)
                        vals[sel] = dloc[idx[sel]]
                        dlx[col0 + lanes] = vals
        ihC_w[c] = _wrap16(ihC)
        dlx2d[c] = dlx.reshape(SxT, P).T.astype(bf16)
        dlxT[c, 0] = dlx.astype(bf16)
        oTh[c] = (np.arange(P, dtype=np.float32)[:, None]
                  == dlx[None, :]).astype(bf16)

    shared = {
        "W1aug": W1aug_b, "W2aug": W2aug_b, "b1rep": b1rep,
        "b2rep": b2rep, "iota": iota, "ident": ident,
        "iotac": np.arange(P, dtype=np.float32).reshape(P, 1),
        "onesk": np.ones((1, P), bf16),
    }
    in_maps = []
    for c in range(NC):
        m = dict(shared)
        m["xTloc"] = np.ascontiguousarray(xT[:, c * NPC:(c + 1) * NPC])
        m["ihsrcC"] = ihC_w[c]
        m["dlx2d"] = dlx2d[c]
        m["dlxT"] = dlxT[c]
        m["oTh"] = oTh[c]
        in_maps.append(m)

    meta = dict(cfg, R1=R1, RG=RG, HC=HC, NPC=NPC, NB=NB, NT=NT, Np=Np,
                CHB=CHB, TPC=TPC, RL2=RL2, Tsum=Tsum, SxT=SxT,
                sb_meta=sb_meta, SBG=SBG)
    return in_maps, meta


# ----------------------------------------------------------------------------
# device program
# ----------------------------------------------------------------------------

def _sub(ap, elem_off, dims):
    return bass.AP(ap.tensor, ap.offset + elem_off, [ap.ap[0], *list(dims)])


def build(meta, nc=None):
    N, F, H, C, CLS = meta["N"], meta["F"], meta["H"], meta["C"], meta["CLS"]
    NC, R1, RG, HC = meta["NC"], meta["R1"], meta["RG"], meta["HC"]
    NPC, NB, NT, Np = meta["NPC"], meta["NB"], meta["NT"], meta["Np"]
    CHB, TPC, RL2 = meta["CHB"], meta["TPC"], meta["RL2"]
    Tsum, SxT = meta["Tsum"], meta["SxT"]
    sb_meta = meta["sb_meta"]
    SBG = meta["SBG"]
    R2 = 4

    f32, bf16, i16 = mybir.dt.float32, mybir.dt.bfloat16, mybir.dt.int16

    if nc is None:
        nc = bacc.Bacc("TRN2", target_bir_lowering=False, debug=False,
                       num_devices=NC, num_swdge_queues=NQUEUE,
                       dynamic_dma_scratch_size=DMA_SCRATCH)

    qrr = [0]

    def gather_split(out_tile, rel, segT, elem, table, ix_tile):
        """Split a segment gather into the fewest <=MAXT-tile calls with
        near-equal sizes (balanced queue drain), round-robin queues."""
        ncalls = (segT + MAXT - 1) // MAXT
        done = 0
        for i in range(ncalls):
            tt = (segT - done + (ncalls - i) - 1) // (ncalls - i)
            r = rel + done
            nc.gpsimd.dma_gather(
                bass.AP(out_tile[:].tensor, out_tile[:].offset + r * elem,
                        [out_tile[:].ap[0], [elem, tt], [1, elem]]),
                table,
                ix_tile[:, r * 8:(r + tt) * 8],
                tt * P, tt * P, elem,
                queue_num=qrr[0] % NQUEUE,
            )
            qrr[0] += 1
            done += tt

    xTl_d = nc.dram_tensor("xTloc", [F, NPC], bf16, kind="ExternalInput")
    W1aug_d = nc.dram_tensor("W1aug", [F, R1 + H], bf16, kind="ExternalInput")
    W2aug_d = nc.dram_tensor("W2aug", [HC, R2], bf16, kind="ExternalInput")
    b1rep_d = nc.dram_tensor("b1rep", [P, HC], bf16, kind="ExternalInput")
    b2rep_d = nc.dram_tensor("b2rep", [P, CLS], f32, kind="ExternalInput")
    iota_d = nc.dram_tensor("iota", [P, P], bf16, kind="ExternalInput")
    ident_d = nc.dram_tensor("ident", [P, P], bf16, kind="ExternalInput")
    ihC_d = nc.dram_tensor("ihsrcC", [P, Tsum * 8], i16, kind="ExternalInput")
    dlx2d_d = nc.dram_tensor("dlx2d", [P, SxT], bf16, kind="ExternalInput")
    dlxT_d = nc.dram_tensor("dlxT", [1, SxT * P], bf16, kind="ExternalInput")
    oTh_d = nc.dram_tensor("oTh", [P, SxT * P], bf16, kind="ExternalInput")
    iotac_d = nc.dram_tensor("iotac", [P, 1], f32, kind="ExternalInput")
    onesk_d = nc.dram_tensor("onesk", [1, P], bf16, kind="ExternalInput")
    out_d = nc.dram_tensor("out", [NPC, CLS], f32, kind="ExternalOutput")

    hloc = nc.dram_tensor("hloc", [NPC, RG], bf16, kind="Internal")
    htabS = nc.dram_tensor("htabS", [Np, RG], bf16, kind="Internal",
                           addr_space="Shared")
    h2loc64 = nc.dram_tensor("h2loc64", [NPC, RL2], f32, kind="Internal")
    h2tab64 = nc.dram_tensor("h2tab64", [Np, RL2], f32, kind="Internal",
                             addr_space="Shared")

    FA = min(P, F)
    FB = F - FA

    with tile.TileContext(nc) as tc:
        with tc.tile_pool(name="const", bufs=1) as cp:
            w1a = cp.tile([FA, R1 + H], bf16)
            nc.sync.dma_start(out=w1a[:], in_=W1aug_d[0:FA, :])
            w1b = cp.tile([FB, R1 + H], bf16)
            nc.sync.dma_start(out=w1b[:], in_=W1aug_d[FA:F, :])
            w2a = cp.tile([P, R2], bf16)
            nc.sync.dma_start(out=w2a[:], in_=W2aug_d[0:P, :])
            w2b = cp.tile([P, R2], bf16)
            nc.sync.dma_start(out=w2b[:], in_=W2aug_d[P:HC, :])
            b1s = cp.tile([P, HC], bf16)
            nc.sync.dma_start(out=b1s[:], in_=b1rep_d[:, :])
            b2s = cp.tile([P, CLS], f32)
            nc.sync.dma_start(out=b2s[:], in_=b2rep_d[:, :])
            iot = cp.tile([P, P], bf16)
            nc.sync.dma_start(out=iot[:], in_=iota_d[:, :])
            idn = cp.tile([P, P], bf16)
            nc.sync.dma_start(out=idn[:], in_=ident_d[:, :])
            dlc = cp.tile([P, SxT], bf16)
            nc.sync.dma_start(out=dlc[:], in_=dlx2d_d[:, :])
            iotc = cp.tile([P, 1], f32)
            nc.sync.dma_start(out=iotc[:], in_=iotac_d[:, :])
            onek = cp.tile([1, P], bf16)
            nc.sync.dma_start(out=onek[:], in_=onesk_d[:, :])
            # persistent SBUF state
            slocS = cp.tile([P, NB * H], bf16)       # a_dst of own nodes
            h2self = cp.tile([P, NB * R2], f32)      # own h2 rows
            vstage = cp.tile([P, NB * R2], f32)      # phase-C accumulators

            # ---------------- mini-pass: h table (own slice) -----------------
            with tc.tile_pool(name="pm", bufs=2) as pm, \
                 tc.tile_pool(name="psm", bufs=4, space="PSUM") as psm:
                for s in range(NPC // (SLAB * P)):          # 7 slabs
                    c0 = s * SLAB * P
                    xa = pm.tile([FA, SLAB * P], bf16, tag="xa")
                    nc.sync.dma_start(out=xa[:], in_=xTl_d[0:FA, c0:c0 + SLAB * P])
                    xb = pm.tile([FB, SLAB * P], bf16, tag="xb")
                    nc.sync.dma_start(out=xb[:], in_=xTl_d[FA:F, c0:c0 + SLAB * P])
                    stg = pm.tile([P, SLAB * RG], bf16, tag="stg")
                    for t in range(SLAB):
                        ph = psm.tile([P, R1 + H], f32, tag="ph")
                        nc.tensor.matmul(out=ph[:], lhsT=xa[:, t * P:(t + 1) * P],
                                         rhs=w1a[:], start=True, stop=False)
                        nc.tensor.matmul(out=ph[:], lhsT=xb[:, t * P:(t + 1) * P],
                                         rhs=w1b[:], start=False, stop=True)
                        nc.vector.tensor_copy(out=stg[:, t * RG:t * RG + R1],
                                              in_=ph[:, :R1])
                        nc.vector.tensor_copy(
                            out=slocS[:, (s * SLAB + t) * H:(s * SLAB + t + 1) * H],
                            in_=ph[:, R1:R1 + H])
                    nc.sync.dma_start(
                        out=bass.AP(hloc, s * SLAB * RG,
                                    [[NB * RG, P], [RG, SLAB], [1, RG]]),
                        in_=stg[:])

            # ---------------- AllGather h table ------------------------------
            nc.gpsimd.collective_compute(
                "AllGather", mybir.AluOpType.bypass,
                replica_groups=[list(range(NC))],
                ins=[hloc[:, :]], outs=[htabS[:, :]])

            # ---------------- Phase B: L1 edge pass --------------------------
            with tc.tile_pool(name="pbg", bufs=3) as pbg, \
                 tc.tile_pool(name="pbo", bufs=2) as pbo, \
                 tc.tile_pool(name="pbb", bufs=3) as pbb, \
                 tc.tile_pool(name="psb", bufs=4, space="PSUM") as psb, \
                 tc.tile_pool(name="psp", bufs=2, space="PSUM") as psp, \
                 tc.tile_pool(name="pst", bufs=1, space="PSUM") as pst, \
                 tc.tile_pool(name="psh", bufs=1, space="PSUM") as psh:
                for sb in sb_meta:
                    base, S, Sx, ohb = sb["base"], sb["S"], sb["Sx"], sb["ohbase"]
                    blist, b0 = sb["blocks"], sb["b0"]
                    nblk = len(blist)
                    g = pbg.tile([P, S * RG], bf16, tag="g")
                    ixs = pbg.tile([P, S * 8], i16, tag="ixs")
                    nc.sync.dma_start(out=ixs[:],
                                      in_=ihC_d[:, base * 8:(base + S) * 8])
                    for q in range(NCHUNK):
                        tb, T = sb["segs"][q]
                        gather_split(g, tb - base, T, RG,
                                     htabS[q * CHB:(q + 1) * CHB, :], ixs)
                    selfh = pbg.tile([P, SBG * RG], bf16, tag="selfh")
                    nc.sync.dma_start(
                        out=selfh[:, :nblk * RG],
                        in_=bass.AP(hloc, b0 * RG,
                                    [[NB * RG, P], [1, nblk * RG]]))
                    # oT: [dst_local, inst_col] one-hot (host-precomputed);
                    # split load so early pad matmuls start at half-load
                    oT = pbo.tile([P, Sx * P], bf16, tag="oT")
                    h1 = (Sx // 2) * P
                    nc.sync.dma_start(out=oT[:, :h1],
                                      in_=oTh_d[:, ohb * P:ohb * P + h1])
                    nc.sync.dma_start(out=oT[:, h1:Sx * P],
                                      in_=oTh_d[:, ohb * P + h1:(ohb + Sx) * P])
                    # per-slot a_dst via oT matmuls -> PSUM [P, S*H]
                    pad = psp.tile([P, S * H], f32, tag="pad")
                    for t in range(S):
                        grp = sb["pad_groups"][t]
                        for gi, (ohc, bi) in enumerate(grp):
                            nc.tensor.matmul(
                                out=pad[:, t * H:(t + 1) * H],
                                lhsT=oT[:, ohc * P:(ohc + 1) * P],
                                rhs=slocS[:, (b0 + bi) * H:(b0 + bi + 1) * H],
                                start=(gi == 0), stop=(gi == len(grp) - 1),
                                skip_group_check=True)
                    # ex = exp(lrelu(asrc+adst))  [P, S*H] f32
                    ex = pbb.tile([P, S * H], f32, tag="ex")
                    nc.vector.tensor_tensor(
                        out=ex[:].rearrange("p (t h) -> p t h", t=S),
                        in0=_sub(g[:], HC, [[RG, S], [1, H]]),
                        in1=pad[:].rearrange("p (t h) -> p t h", t=S),
                        op=mybir.AluOpType.add)
                    tmp = pbb.tile([P, S * H], f32, tag="tmp")
                    nc.vector.tensor_scalar_mul(out=tmp[:], in0=ex[:], scalar1=NEG)
                    nc.vector.tensor_tensor(out=ex[:], in0=ex[:], in1=tmp[:],
                                            op=mybir.AluOpType.max)
                    nc.scalar.activation(out=ex[:], in_=ex[:],
                                         func=mybir.ActivationFunctionType.Exp)
                    exb = pbb.tile([P, S * H], bf16, tag="exb")
                    nc.vector.tensor_copy(out=exb[:], in_=ex[:])
                    # msg in-place: cols 0:HC *= ex ; cols HC:HC+H = ex
                    nc.vector.tensor_tensor(
                        out=_sub(g[:], 0, [[RG, S], [C, H], [1, C]]),
                        in0=_sub(g[:], 0, [[RG, S], [C, H], [1, C]]),
                        in1=_sub(exb[:], 0, [[H, S], [1, H], [0, C]]),
                        op=mybir.AluOpType.mult)
                    nc.vector.tensor_copy(
                        out=_sub(g[:], HC, [[RG, S], [1, H]]),
                        in_=exb[:].rearrange("p (t h) -> p t h", t=S))
                    # oh: [slot, inst_col] one-hot
                    oh = pbo.tile([P, Sx * P], bf16, tag="oh")
                    nc.vector.tensor_tensor(
                        out=oh[:].rearrange("p (t q) -> p t q", t=Sx),
                        in0=_sub(dlc[:], ohb, [[1, Sx], [0, P]]),
                        in1=_sub(iot[:], 0, [[0, Sx], [1, P]]),
                        op=mybir.AluOpType.is_equal)
                    # self-loop messages (batched over blocks)
                    lS = pbb.tile([P, SBG * H], f32, tag="lS")
                    nc.vector.tensor_tensor(
                        out=lS[:, :nblk * H].rearrange("p (b h) -> p b h", b=nblk),
                        in0=_sub(selfh[:], HC, [[RG, nblk], [1, H]]),
                        in1=_sub(slocS[:], b0 * H, [[H, nblk], [1, H]]),
                        op=mybir.AluOpType.add)
                    tS = pbb.tile([P, SBG * H], f32, tag="tS")
                    nc.vector.tensor_scalar_mul(out=tS[:], in0=lS[:], scalar1=NEG)
                    nc.vector.tensor_tensor(out=lS[:], in0=lS[:], in1=tS[:],
                                            op=mybir.AluOpType.max)
                    nc.scalar.activation(out=lS[:], in_=lS[:],
                                         func=mybir.ActivationFunctionType.Exp)
                    selfm = pbb.tile([P, SBG * (R1 + 4)], bf16, tag="selfm")
                    RS = R1 + 4
                    nc.vector.tensor_tensor(
                        out=_sub(selfm[:], 0, [[RS, nblk], [C, H], [1, C]]),
                        in0=_sub(selfh[:], 0, [[RG, nblk], [C, H], [1, C]]),
                        in1=_sub(lS[:], 0, [[H, nblk], [1, H], [0, C]]),
                        op=mybir.AluOpType.mult)
                    nc.vector.tensor_copy(
                        out=_sub(selfm[:], HC, [[RS, nblk], [1, H]]),
                        in_=lS[:, :nblk * H].rearrange("p (b h) -> p b h", b=nblk))
                    # per-block aggregation + batched epilogue
                    psoS = pbb.tile([P, SBG * RS], f32, tag="psoS")
                    for bi in range(nblk):
                        runs = sb["agg"][bi]
                        pso = psb.tile([P, R1], f32, tag="pso")
                        for ri, (t_rel, ohc) in enumerate(runs):
                            nc.tensor.matmul(
                                out=pso[:],
                                lhsT=oh[:, ohc * P:(ohc + 1) * P],
                                rhs=g[:, t_rel * RG:t_rel * RG + R1],
                                start=(ri == 0), stop=(ri == len(runs) - 1))
                        nc.vector.tensor_tensor(
                            out=psoS[:, bi * RS:bi * RS + R1],
                            in0=pso[:],
                            in1=selfm[:, bi * RS:bi * RS + R1],
                            op=mybir.AluOpType.add)
                    den = pbb.tile([P, SBG * H], f32, tag="den")
                    nc.vector.tensor_scalar_max(
                        out=den[:, :nblk * H].rearrange("p (b h) -> p b h", b=nblk),
                        in0=_sub(psoS[:], HC, [[RS, nblk], [1, H]]),
                        scalar1=1e-20)
                    rde = pbb.tile([P, SBG * H], f32, tag="rde")
                    nc.vector.reciprocal(out=rde[:], in_=den[:])
                    o1 = pbb.tile([P, SBG * HC], bf16, tag="o1")
                    nc.vector.tensor_tensor(
                        out=o1[:].rearrange("p (b h c) -> p b h c", b=SBG, h=H),
                        in0=_sub(psoS[:], 0, [[RS, SBG], [C, H], [1, C]]),
                        in1=_sub(rde[:], 0, [[H, SBG], [1, H], [0, C]]),
                        op=mybir.AluOpType.mult)
                    nc.vector.tensor_tensor(
                        out=o1[:].rearrange("p (b k) -> p b k", b=SBG),
                        in0=o1[:].rearrange("p (b k) -> p b k", b=SBG),
                        in1=_sub(b1s[:], 0, [[0, SBG], [1, HC]]),
                        op=mybir.AluOpType.add)
                    nc.scalar.activation(out=o1[:], in_=o1[:],
                                         func=mybir.ActivationFunctionType.Relu)
                    # h2 = relu(o1) @ W2aug via PE transposes
                    ptr = pst.tile([P, 2 * SBG * P], bf16, tag="ptr")
                    for bi in range(nblk):
                        for k in range(2):
                            nc.tensor.transpose(
                                out=ptr[:, (bi * 2 + k) * P:(bi * 2 + k + 1) * P],
                                in_=o1[:, bi * HC + k * P:bi * HC + (k + 1) * P],
                                identity=idn[:])
                    rT = pbb.tile([P, 2 * SBG * P], bf16, tag="rT")
                    nc.vector.tensor_copy(out=rT[:, :nblk * 2 * P],
                                          in_=ptr[:, :nblk * 2 * P])
                    ph2 = psh.tile([P, SBG * R2], f32, tag="ph2")
                    for bi in range(nblk):
                        nc.tensor.matmul(out=ph2[:, bi * R2:(bi + 1) * R2],
                                         lhsT=rT[:, bi * 2 * P:(bi * 2 + 1) * P],
                                         rhs=w2a[:], start=True, stop=False,
                                         skip_group_check=True)
                        nc.tensor.matmul(out=ph2[:, bi * R2:(bi + 1) * R2],
                                         lhsT=rT[:, (bi * 2 + 1) * P:(bi * 2 + 2) * P],
                                         rhs=w2b[:], start=False, stop=True,
                                         skip_group_check=True)
                    nc.vector.tensor_copy(out=h2self[:, b0 * R2:(b0 + nblk) * R2],
                                          in_=ph2[:, :nblk * R2])
                    h2st = pbb.tile([P, SBG * RL2], f32, tag="h2st")
                    nc.vector.tensor_copy(
                        out=_sub(h2st[:], 0, [[RL2, nblk], [1, R2]]),
                        in_=ph2[:, :nblk * R2].rearrange("p (b r) -> p b r", b=nblk))
                    nc.sync.dma_start(
                        out=bass.AP(h2loc64, b0 * RL2,
                                    [[NB * RL2, P], [1, nblk * RL2]]),
                        in_=h2st[:, :nblk * RL2])

            # ---------------- AllGather ---------------------------------------
            nc.gpsimd.collective_compute(
                "AllGather", mybir.AluOpType.bypass,
                replica_groups=[list(range(NC))],
                ins=[h2loc64[:, :]], outs=[h2tab64[:, :]])

            # ---------------- Phase C: L2 edge pass --------------------------
            with tc.tile_pool(name="pcg", bufs=4) as pcg, \
                 tc.tile_pool(name="pco", bufs=3) as pco, \
                 tc.tile_pool(name="pcb", bufs=2) as pcb, \
                 tc.tile_pool(name="psc", bufs=4, space="PSUM") as psc, \
                 tc.tile_pool(name="psp2", bufs=2, space="PSUM") as psp2, \
                 tc.tile_pool(name="psk2", bufs=2, space="PSUM") as psk2:
                for sb in sb_meta:
                    base, S, Sx, ohb = sb["base"], sb["S"], sb["Sx"], sb["ohbase"]
                    blist, b0 = sb["blocks"], sb["b0"]
                    nblk = len(blist)
                    g2 = pcg.tile([P, S * RL2], f32, tag="g2")
                    ixs = pcg.tile([P, S * 8], i16, tag="ixs2")
                    nc.sync.dma_start(out=ixs[:],
                                      in_=ihC_d[:, base * 8:(base + S) * 8])
                    for q in range(NCHUNK):
                        tb, T = sb["segs"][q]
                        gather_split(g2, tb - base, T, RL2,
                                     h2tab64[q * CHB:(q + 1) * CHB, :], ixs)
                    dlT = pcg.tile([1, Sx * P], bf16, tag="dlT2")
                    nc.sync.dma_start(out=dlT[:],
                                      in_=dlxT_d[0:1, ohb * P:(ohb + Sx) * P])
                    oT = pco.tile([P, Sx * P], bf16, tag="oT2")
                    for st in range(0, Sx * P, 512):
                        w = min(512, Sx * P - st)
                        stp = psk2.tile([P, 512], f32, tag="stp2")
                        nc.tensor.matmul(out=stp[:, :w], lhsT=onek[:],
                                         rhs=dlT[0:1, st:st + w],
                                         start=True, stop=True)
                        nc.vector.tensor_tensor(
                            out=oT[:, st:st + w],
                            in0=iotc[:, 0:1].to_broadcast([P, w]),
                            in1=stp[:, :w],
                            op=mybir.AluOpType.is_equal)
                    adw2 = pcb.tile([P, SBG], bf16, tag="adw2")
                    nc.vector.tensor_copy(
                        out=adw2[:, :nblk],
                        in_=_sub(h2self[:], b0 * R2 + 3, [[R2, nblk]]))
                    pad2 = psp2.tile([P, S], f32, tag="pad2")
                    for t in range(S):
                        grp = sb["pad_groups"][t]
                        for gi, (ohc, bi) in enumerate(grp):
                            nc.tensor.matmul(
                                out=pad2[:, t:t + 1],
                                lhsT=oT[:, ohc * P:(ohc + 1) * P],
                                rhs=adw2[:, bi:bi + 1],
                                start=(gi == 0), stop=(gi == len(grp) - 1),
                                skip_group_check=True)
                    ex2 = pcb.tile([P, S], f32, tag="ex2")
                    nc.vector.tensor_tensor(
                        out=ex2[:],
                        in0=_sub(g2[:], CLS, [[RL2, S]]),
                        in1=pad2[:],
                        op=mybir.AluOpType.add)
                    tm2 = pcb.tile([P, S], f32, tag="tm2")
                    nc.vector.tensor_scalar_mul(out=tm2[:], in0=ex2[:], scalar1=NEG)
                    nc.vector.tensor_tensor(out=ex2[:], in0=ex2[:], in1=tm2[:],
                                            op=mybir.AluOpType.max)
                    nc.scalar.activation(out=ex2[:], in_=ex2[:],
                                         func=mybir.ActivationFunctionType.Exp)
                    m2 = pcb.tile([P, S * R2], bf16, tag="m2")
                    nc.vector.tensor_tensor(
                        out=_sub(m2[:], 0, [[R2, S], [1, CLS]]),
                        in0=_sub(g2[:], 0, [[RL2, S], [1, CLS]]),
                        in1=_sub(ex2[:], 0, [[1, S], [0, CLS]]),
                        op=mybir.AluOpType.mult)
                    nc.vector.tensor_copy(
                        out=_sub(m2[:], CLS, [[R2, S], [1, 2]]),
                        in_=_sub(ex2[:], 0, [[1, S], [0, 2]]))
                    oh = pco.tile([P, Sx * P], bf16, tag="oh2")
                    nc.vector.tensor_tensor(
                        out=oh[:].rearrange("p (t q) -> p t q", t=Sx),
                        in0=_sub(dlc[:], ohb, [[1, Sx], [0, P]]),
                        in1=_sub(iot[:], 0, [[0, Sx], [1, P]]),
                        op=mybir.AluOpType.is_equal)
                    # self-loop L2 messages
                    l2S = pcb.tile([P, SBG], f32, tag="l2S")
                    nc.vector.tensor_tensor(
                        out=l2S[:, :nblk],
                        in0=_sub(h2self[:], b0 * R2 + 2, [[R2, nblk]]),
                        in1=_sub(h2self[:], b0 * R2 + 3, [[R2, nblk]]),
                        op=mybir.AluOpType.add)
                    t2S = pcb.tile([P, SBG], f32, tag="t2S")
                    nc.vector.tensor_scalar_mul(out=t2S[:], in0=l2S[:], scalar1=NEG)
                    nc.vector.tensor_tensor(out=l2S[:], in0=l2S[:], in1=t2S[:],
                                            op=mybir.AluOpType.max)
                    nc.scalar.activation(out=l2S[:], in_=l2S[:],
                                         func=mybir.ActivationFunctionType.Exp)
                    sm2 = pcb.tile([P, SBG * R2], f32, tag="sm2")
                    nc.vector.tensor_tensor(
                        out=_sub(sm2[:], 0, [[R2, nblk], [1, CLS]]),
                        in0=_sub(h2self[:], b0 * R2, [[R2, nblk], [1, CLS]]),
                        in1=_sub(l2S[:], 0, [[1, nblk], [0, CLS]]),
                        op=mybir.AluOpType.mult)
                    nc.vector.tensor_copy(
                        out=_sub(sm2[:], CLS, [[R2, nblk], [1, 2]]),
                        in_=_sub(l2S[:], 0, [[1, nblk], [0, 2]]))
                    ps2 = psc.tile([P, SBG * R2], f32, tag="ps2")
                    for bi in range(nblk):
                        runs = sb["agg"][bi]
                        for ri, (t_rel, ohc) in enumerate(runs):
                            nc.tensor.matmul(
                                out=ps2[:, bi * R2:(bi + 1) * R2],
                                lhsT=oh[:, ohc * P:(ohc + 1) * P],
                                rhs=m2[:, t_rel * R2:(t_rel + 1) * R2],
                                start=(ri == 0), stop=(ri == len(runs) - 1),
                                skip_group_check=True)
                    nc.vector.tensor_tensor(
                        out=vstage[:, b0 * R2:(b0 + nblk) * R2],
                        in0=ps2[:, :nblk * R2],
                        in1=sm2[:, :nblk * R2],
                        op=mybir.AluOpType.add)

                # ------------- final: normalize + log-softmax ----------------
                den2 = pcb.tile([P, NB], f32, tag="den2")
                nc.vector.tensor_scalar_max(out=den2[:],
                                            in0=_sub(vstage[:], CLS, [[R2, NB]]),
                                            scalar1=1e-20)
                rd2 = pcb.tile([P, NB], f32, tag="rd2")
                nc.vector.reciprocal(out=rd2[:], in_=den2[:])
                v = pcb.tile([P, NB * CLS], f32, tag="v")
                nc.vector.tensor_tensor(
                    out=v[:].rearrange("p (b k) -> p b k", b=NB),
                    in0=_sub(vstage[:], 0, [[R2, NB], [1, CLS]]),
                    in1=_sub(rd2[:], 0, [[1, NB], [0, CLS]]),
                    op=mybir.AluOpType.mult)
                nc.vector.tensor_tensor(
                    out=v[:].rearrange("p (b k) -> p b k", b=NB),
                    in0=v[:].rearrange("p (b k) -> p b k", b=NB),
                    in1=_sub(b2s[:], 0, [[0, NB], [1, CLS]]),
                    op=mybir.AluOpType.add)
                mx = pcb.tile([P, NB], f32, tag="mx")
                nc.vector.tensor_tensor(out=mx[:],
                                        in0=_sub(v[:], 0, [[CLS, NB]]),
                                        in1=_sub(v[:], 1, [[CLS, NB]]),
                                        op=mybir.AluOpType.max)
                u = pcb.tile([P, NB * CLS], f32, tag="u")
                nc.vector.tensor_tensor(
                    out=u[:].rearrange("p (b k) -> p b k", b=NB),
                    in0=v[:].rearrange("p (b k) -> p b k", b=NB),
                    in1=_sub(mx[:], 0, [[1, NB], [0, CLS]]),
                    op=mybir.AluOpType.subtract)
                nc.scalar.activation(out=u[:], in_=u[:],
                                     func=mybir.ActivationFunctionType.Exp)
                sm = pcb.tile([P, NB], f32, tag="sm")
                nc.vector.tensor_tensor(out=sm[:],
                                        in0=_sub(u[:], 0, [[CLS, NB]]),
                                        in1=_sub(u[:], 1, [[CLS, NB]]),
                                        op=mybir.AluOpType.add)
                ls = pcb.tile([P, NB], f32, tag="ls")
                nc.scalar.activation(out=ls[:], in_=sm[:],
                                     func=mybir.ActivationFunctionType.Ln)
                nc.vector.tensor_tensor(out=ls[:], in0=ls[:], in1=mx[:],
                                        op=mybir.AluOpType.add)
                res = pcb.tile([P, NB * CLS], f32, tag="res")
                nc.vector.tensor_tensor(
                    out=res[:].rearrange("p (b k) -> p b k", b=NB),
                    in0=v[:].rearrange("p (b k) -> p b k", b=NB),
                    in1=_sub(ls[:], 0, [[1, NB], [0, CLS]]),
                    op=mybir.AluOpType.subtract)
                nc.sync.dma_start(
                    out=bass.AP(out_d, 0, [[NB * CLS, P], [1, NB * CLS]]),
                    in_=res[:])
    nc.finalize()
    return nc


def install_ntff_hook(so_path="/opt/axon/libaxon_pjrt.so"):
    import types
    import ctypes
    import contextlib
    import antenv

    if getattr(antenv, "axon_hooks", None) is not None:
        return
    lib = ctypes.CDLL(so_path)
    if not hasattr(lib, "axon_start_nrt_profile"):
        return
    lib.axon_start_nrt_profile.argtypes = [ctypes.POINTER(ctypes.c_int64),
                                           ctypes.c_size_t]
    lib.axon_start_nrt_profile.restype = ctypes.c_int64
    lib.axon_stop_nrt_profile.argtypes = [ctypes.c_char_p]
    lib.axon_stop_nrt_profile.restype = ctypes.c_int64

    @contextlib.contextmanager
    def _hook(output_dir, device_ids):
        import jax
        jax.devices()
        if device_ids:
            ids = (ctypes.c_int64 * len(device_ids))(*device_ids)
            rc = lib.axon_start_nrt_profile(ids, len(device_ids))
        else:
            rc = lib.axon_start_nrt_profile(None, 0)
        if rc != 0:
            raise RuntimeError(f"axon_start_nrt_profile rc={rc}")
        try:
            yield
        finally:
            n = lib.axon_stop_nrt_profile(str(output_dir).encode())
            print(f"ntff profile: {n} file(s) written to {output_dir}")

    mod = types.ModuleType("antenv.axon_hooks")
    _reg = [_hook]
    mod.set_axon_ntff_profile_hook = lambda h: _reg.__setitem__(0, h)
    mod.get_axon_ntff_profile_hook = lambda: _reg[0]
    sys.modules["antenv.axon_hooks"] = mod
    antenv.axon_hooks = mod


def run(inputs, cfg, trace=False, **kwargs):
    if trace:
        install_ntff_hook()
    in_maps, meta = prep(inputs, cfg)
    nc = build(meta)
    res = bass_utils.run_bass_kernel_spmd(
        nc, in_maps, core_ids=list(range(cfg["NC"])), trace=trace, **kwargs)
    NPC, NB, N = meta["NPC"], meta["NB"], meta["N"]
    parts = []
    for c in range(cfg["NC"]):
        r = np.asarray(res.results[c]["out"])          # [NPC, 2], (p, b) order
        r = r.reshape(P, NB, cfg["CLS"]).transpose(1, 0, 2).reshape(NPC, cfg["CLS"])
        parts.append(r)
    out = np.concatenate(parts, axis=0)[:N]
    return out, res


# ----------------------------------------------------------------------------
# harness entry point
# ----------------------------------------------------------------------------

_CFG = dict(N=100000, F=165, H=4, C=64, CLS=2, NC=8, SBG=4)


def kernel(**inputs):
    """Full (unsharded) inputs -> full [N, 2] float32 log-softmax output.

    Shards edges by destination-node range across the 8 NeuronCores,
    compiles and runs the Bass/Tile kernel via run_bass_kernel_spmd,
    and reassembles the per-core output slices.
    """
    out, _ = run(inputs, _CFG, trace=False)
    return np.ascontiguousarray(out.astype(np.float32))
